# revision 9
# baseline (speedup 1.0000x reference)
"""CausalWanSelfAttention on 8 trn2 NeuronCores (Bass/Tile, SPMD).

Sharding: 4 head-groups (3 heads each) x 2 query-halves (780 q-rows per frame).
The frame mask (F=3, sink=1, local=2) is exactly frame-block-causal, so
attention is dense per (qframe, kframe<=qframe) block.

v2 pipeline per device:
  One pass over x computes K (feature-major, rope fused into PSUM evac) AND
  V (token-major) from shared x tiles; Q proj (core's 2340 tokens) likewise
  rope-fused.  Sum-of-squares partials accumulate in SBUF row tiles and go
  out as TWO mask-free AllReduces over the same-query-half 4-core groups
  ({0,2,4,6}/{1,3,5,7}): AR#1 = [ss_q | ss_k(0:3456)] issued before the K/V
  tail (tokens 3456:4680) so the tail + frame-0/1 attention hide AR latency;
  AR#2 = ss_k(3456:) hidden under frame-0/1 attention.  rms_k folds into the
  exp scale (per-partition in transposed-score layout), rms_q (and 1/sqrt(d))
  folds into qT.  Transposed-score flash attention without max-subtraction;
  softmax denominators via ones-column matmuls over quad-added exp tiles;
  per-core partial O-projection (fp16) summed on the host.
"""
import sys
sys.path.insert(0, '/opt/trn_rl_repo')

import numpy as np
import ml_dtypes

import concourse.bass as bass
import concourse.mybir as mybir
import concourse.tile as tile
from concourse import bacc, bass_utils

F32 = mybir.dt.float32
F16 = mybir.dt.float16
BF16 = mybir.dt.bfloat16
BF = ml_dtypes.bfloat16

L, D, NH, HD = 4680, 1536, 12, 128
NFR, FRT = 3, 1560          # frames, tokens per frame
QH = 780                    # q rows per (core, frame)
QW = 3 * QH                 # 2340 q tokens per core
KC = 12                     # contraction chunks of 128
TG = 384                    # token group for k/v proj (12 full + 72 runt)
QTG = 468                   # token group for q proj (5 exact)
KSPLIT = 3456               # k tokens covered by AR#1 (groups 0..8)
KCOL1 = KSPLIT // 128       # 27
LPAD = 4736                 # 37*128
KCOL2 = (LPAD - KSPLIT) // 128  # 10 (last col has 72 valid)
CC1 = QW + KSPLIT           # 5796 f32
CC2 = LPAD - KSPLIT         # 1280 f32 (tail zero-padded)
EPS = 1e-6
ISD = float(1.0 / np.sqrt(HD))

_nc_cache = {}


def _rope_tables(freqs):
    """cos/sin [64, L] float32 per rope pair, matching reference _rope_table."""
    c = freqs.shape[1]           # 64
    s0 = c - 2 * (c // 3)        # 22
    s1 = c // 3                  # 21
    Fr, H, W = NFR, 30, 52
    fr = np.zeros((Fr, H, W, c, 2), np.float32)
    fr[:, :, :, :s0] = freqs[:Fr, :s0].reshape(Fr, 1, 1, s0, 2)
    fr[:, :, :, s0:s0 + s1] = freqs[:H, s0:s0 + s1].reshape(1, H, 1, s1, 2)
    fr[:, :, :, s0 + s1:] = freqs[:W, s0 + s1:].reshape(1, 1, W, s1, 2)
    fr = fr.reshape(L, c, 2)
    return fr[:, :, 0].T.copy(), fr[:, :, 1].T.copy()  # [64, L] each


def _build(has_bias, has_gk):
    nc = bacc.Bacc(trn_type="TRN2", debug=False, num_devices=8)
    ExtIn = dict(kind="ExternalInput")
    T = {}
    T["xT"] = nc.dram_tensor("xT", [KC, 128, L], BF16, **ExtIn)
    T["xTq"] = nc.dram_tensor("xTq", [KC, 128, QW], BF16, **ExtIn)
    T["wqT"] = nc.dram_tensor("wqT", [KC, 128, 384], BF16, **ExtIn)
    T["wkT"] = nc.dram_tensor("wkT", [KC, 128, 384], BF16, **ExtIn)
    T["wvT"] = nc.dram_tensor("wvT", [KC, 128, 384], BF16, **ExtIn)
    T["woT"] = nc.dram_tensor("woT", [3, KC, 128, 128], BF16, **ExtIn)
    T["cosk"] = nc.dram_tensor("cosk", [64, L], BF16, **ExtIn)
    T["sink"] = nc.dram_tensor("sink", [64, L], BF16, **ExtIn)
    T["cosq"] = nc.dram_tensor("cosq", [64, QW], BF16, **ExtIn)
    T["sinq"] = nc.dram_tensor("sinq", [64, QW], BF16, **ExtIn)
    T["gq_d"] = nc.dram_tensor("gq_d", [128, 3], F32, **ExtIn)
    if has_gk:
        T["gk_d"] = nc.dram_tensor("gk_d", [128, 3], F32, **ExtIn)
    if has_bias:
        T["bqkv"] = nc.dram_tensor("bqkv", [1, 3, 384], BF16, **ExtIn)
    T["outOT"] = nc.dram_tensor("outOT", [D, QW], F16, kind="ExternalOutput")
    T["cc1_in"] = nc.dram_tensor("cc1_in", [1, CC1], F32)
    T["cc1_out"] = nc.dram_tensor("cc1_out", [1, CC1], F32)
    T["cc2_in"] = nc.dram_tensor("cc2_in", [1, CC2], F32)
    T["cc2_out"] = nc.dram_tensor("cc2_out", [1, CC2], F32)

    with tile.TileContext(nc) as tc:
        _emit(nc, tc, T, has_bias, has_gk)
    nc.compile()
    return nc


def _emit(nc, tc, T, has_bias, has_gk):
    from contextlib import ExitStack
    RG = [[0, 1, 2, 3], [4, 5, 6, 7]]
    es = ExitStack()
    with es:
        keep = es.enter_context(tc.tile_pool(name="keep", bufs=1))
        rows = es.enter_context(tc.tile_pool(name="rows", bufs=1))

        gq_sb = keep.tile([128, 3], F32, tag="gq")
        nc.sync.dma_start(out=gq_sb, in_=T["gq_d"].ap())
        if has_gk:
            gk_sb = keep.tile([128, 3], F32, tag="gk")
            nc.sync.dma_start(out=gk_sb, in_=T["gk_d"].ap())
        ones_sb = keep.tile([128, 1], BF16, tag="ones")
        nc.vector.memset(ones_sb, 1.0)
        eps_sb = keep.tile([128, 1], F32, tag="eps")
        nc.vector.memset(eps_sb, EPS)
        if has_bias:
            b_sb = keep.tile([1, 3, 384], BF16, tag="bqkv")
            nc.sync.dma_start(out=b_sb, in_=T["bqkv"].ap())
            onesrow = keep.tile([1, 512], BF16, tag="onesrow")
            nc.vector.memset(onesrow, 1.0)

        kT = [keep.tile([128, L], BF16, tag=f"kT{h}", name=f"kT{h}")
              for h in range(3)]
        qraw = [keep.tile([128, QW], BF16, tag=f"qraw{h}", name=f"qraw{h}")
                for h in range(3)]
        qT = [keep.tile([128, QW], BF16, tag=f"qT{h}", name=f"qT{h}")
              for h in range(3)]
        ntok_tiles = (L + 127) // 128  # 37
        v_sb = [keep.tile([min(128, L - i * 128), 384], BF16, tag=f"v{i}",
                          name=f"v{i}")
                for i in range(ntok_tiles)]


        # =============== P1: projections + rope + ss partials ===============
        with tc.tile_pool(name="pw", bufs=1) as pw, \
             tc.tile_pool(name="pxk", bufs=2) as pxk, \
             tc.tile_pool(name="ptab", bufs=2) as ptab, \
             tc.tile_pool(name="tmp", bufs=1) as tmp, \
             tc.tile_pool(name="tmp2", bufs=2) as tmp2, \
             tc.tile_pool(name="ppk", bufs=2, space="PSUM") as ppk, \
             tc.tile_pool(name="ppv", bufs=2, space="PSUM") as ppv, \
             tc.tile_pool(name="pps", bufs=2, space="PSUM") as pps:
            wk_sb = pw.tile([128, KC, 384], BF16, tag="wk")
            nc.sync.dma_start(out=wk_sb, in_=T["wkT"].ap().rearrange("c p n -> p c n"))
            wv_sb = pw.tile([128, KC, 384], BF16, tag="wv")
            nc.sync.dma_start(out=wv_sb, in_=T["wvT"].ap().rearrange("c p n -> p c n"))
            wq_sb = pw.tile([128, KC, 384], BF16, tag="wq")
            nc.sync.dma_start(out=wq_sb, in_=T["wqT"].ap().rearrange("c p n -> p c n"))

            def rope_evac(psum, cos_sl, sin_sl, dst, col0, n, g_sl):
                t1 = tmp.tile([64, QTG], F32, tag="t1")
                t2 = tmp.tile([64, QTG], F32, tag="t2")
                t3 = tmp.tile([64, QTG], F32, tag="t3")
                t4 = tmp.tile([64, QTG], F32, tag="t4")
                nc.vector.tensor_mul(t1[:, :n], psum[0:64, :n], cos_sl)
                nc.vector.tensor_mul(t2[:, :n], psum[64:128, :n], sin_sl)
                nc.vector.tensor_mul(t3[:, :n], psum[0:64, :n], sin_sl)
                nc.vector.tensor_mul(t4[:, :n], psum[64:128, :n], cos_sl)
                nc.vector.tensor_sub(dst[0:64, col0:col0 + n], t1[:, :n], t2[:, :n])
                nc.vector.tensor_add(dst[64:128, col0:col0 + n], t3[:, :n], t4[:, :n])
                # ss partial from the roped (pre-gain) values
                sq = tmp2.tile([128, QTG], BF16, tag="sq", bufs=8)
                nc.scalar.square(sq[:, :n], dst[:, col0:col0 + n])
                if g_sl is not None:
                    gtmp = tmp2.tile([128, QTG], BF16, tag="gtmp")
                    nc.vector.tensor_scalar_mul(gtmp[:, :n],
                                                dst[:, col0:col0 + n], g_sl)
                    nc.scalar.copy(out=dst[:, col0:col0 + n], in_=gtmp[:, :n])
                return sq

            # Deferred ss emission: the ones-matmul waits on Square(ACT); by
            # lagging one token-group the PE queue never stalls on it.
            ss_pending = []

            def flush_ss():
                for sqs, ss_ps, n, dram, off in ss_pending:
                    for h, sq in enumerate(sqs):
                        nc.tensor.matmul(ss_ps[:, :n], ones_sb, sq[:, :n],
                                         start=(h == 0), stop=(h == 2))
                    st = tmp2.tile([1, QTG], F32, tag="ssst")
                    nc.vector.tensor_copy(st[:, :n], ss_ps[:, :n])
                    nc.sync.dma_start(out=dram.ap()[:, off:off + n],
                                      in_=st[:, :n])
                ss_pending.clear()

            def kv_group(tg):
                c0 = tg * TG
                n = min(TG, L - c0)
                xk = pxk.tile([128, KC, QTG], BF16, tag="xk")
                nc.sync.dma_start(
                    out=xk[:, :, :n],
                    in_=T["xT"].ap()[:, :, c0:c0 + n].rearrange("c p n -> p c n"))
                ck = ptab.tile([64, QTG], BF16, tag="cs")
                nc.sync.dma_start(out=ck[:, :n], in_=T["cosk"].ap()[:, c0:c0 + n])
                sk = ptab.tile([64, QTG], BF16, tag="sn")
                nc.sync.dma_start(out=sk[:, :n], in_=T["sink"].ap()[:, c0:c0 + n])
                ss_ps = pps.tile([1, QTG], F32, tag="pss")
                sqs = []
                for h in range(3):
                    psk = ppk.tile([128, QTG], F32, tag="pk", bufs=3)
                    for kc in range(KC):
                        nc.tensor.matmul(psk[:, :n],
                                         wk_sb[:, kc, h * 128:(h + 1) * 128],
                                         xk[:, kc, :n], start=(kc == 0),
                                         stop=(not has_bias and kc == KC - 1))
                    if has_bias:
                        nc.tensor.matmul(psk[:, :n],
                                         b_sb[:, 1, h * 128:(h + 1) * 128],
                                         onesrow[:, :n], start=False, stop=True)
                    g_sl = gk_sb[:, h:h + 1] if has_gk else None
                    sqs.append(rope_evac(psk, ck[:, :n], sk[:, :n], kT[h], c0, n,
                                         g_sl))
                # V proj from the same x tiles (token-major)
                for j in range(3):
                    vi = tg * 3 + j
                    if vi >= ntok_tiles or vi * 128 >= c0 + n:
                        break
                    rsz = v_sb[vi].shape[0]
                    j0 = vi * 128 - c0
                    psv = ppv.tile([128, 384], F32, tag="pv")
                    for kc in range(KC):
                        nc.tensor.matmul(psv[:rsz, :], xk[:, kc, j0:j0 + rsz],
                                         wv_sb[:, kc, :], start=(kc == 0),
                                         stop=(not has_bias and kc == KC - 1))
                    if has_bias:
                        nc.tensor.matmul(psv[:rsz, :], onesrow[:, :rsz],
                                         b_sb[:, 2, :], start=False, stop=True)
                    nc.vector.tensor_copy(v_sb[vi], psv[:rsz, :])
                flush_ss()
                if c0 < KSPLIT:
                    ss_pending.append((sqs, ss_ps, n, T["cc1_in"], QW + c0))
                else:
                    ss_pending.append((sqs, ss_ps, n, T["cc2_in"], c0 - KSPLIT))

            ng = (L + TG - 1) // TG  # 13
            ng1 = KSPLIT // TG       # 9 groups before AR#1
            for tg in range(ng1):
                kv_group(tg)

            # --- Q projection (core's 2340 tokens) over 5 groups of 468
            for tg in range(QW // QTG):
                c0 = tg * QTG
                n = QTG
                xq = pxk.tile([128, KC, QTG], BF16, tag="xk")
                nc.sync.dma_start(
                    out=xq,
                    in_=T["xTq"].ap()[:, :, c0:c0 + n].rearrange("c p n -> p c n"))
                cq = ptab.tile([64, QTG], BF16, tag="cs")
                nc.sync.dma_start(out=cq, in_=T["cosq"].ap()[:, c0:c0 + n])
                sq_t = ptab.tile([64, QTG], BF16, tag="sn")
                nc.sync.dma_start(out=sq_t, in_=T["sinq"].ap()[:, c0:c0 + n])
                ss_ps = pps.tile([1, QTG], F32, tag="pss")
                sqs = []
                for h in range(3):
                    psq = ppk.tile([128, QTG], F32, tag="pk", bufs=3)
                    for kc in range(KC):
                        nc.tensor.matmul(psq,
                                         wq_sb[:, kc, h * 128:(h + 1) * 128],
                                         xq[:, kc, :], start=(kc == 0),
                                         stop=(not has_bias and kc == KC - 1))
                    if has_bias:
                        nc.tensor.matmul(psq, b_sb[:, 0, h * 128:(h + 1) * 128],
                                         onesrow[:, :n], start=False, stop=True)
                    sqs.append(rope_evac(psq, cq, sq_t, qraw[h], c0, n, None))
                flush_ss()
                ss_pending.append((sqs, ss_ps, n, T["cc1_in"], c0))
            flush_ss()

            # =============== AR#1: ss_q + ss_k[0:KSPLIT] ====================
            nc.gpsimd.collective_compute(
                "AllReduce", mybir.AluOpType.add, replica_groups=RG,
                ins=[T["cc1_in"].ap().opt()], outs=[T["cc1_out"].ap().opt()])

            # --- K/V tail (tokens KSPLIT:L) overlaps AR#1
            for tg in range(ng1, ng):
                kv_group(tg)
            flush_ss()

            # =============== AR#2: ss_k[KSPLIT:] ============================
            zr = tmp2.tile([1, 64], F32, tag="zr")
            nc.vector.memset(zr, 0.0)
            nc.sync.dma_start(out=T["cc2_in"].ap()[:, L - KSPLIT:CC2],
                              in_=zr[:, :CC2 - (L - KSPLIT)])
            nc.gpsimd.collective_compute(
                "AllReduce", mybir.AluOpType.add, replica_groups=RG,
                ins=[T["cc2_in"].ap().opt()], outs=[T["cc2_out"].ap().opt()])

            # =============== rms from AR#1 ==================================
            # q: row [1, QW] -> rsqrt -> broadcast -> fold gq into qT
            ssq_all = rows.tile([1, QW], F32, tag="ssqall")
            nc.sync.dma_start(out=ssq_all, in_=T["cc1_out"].ap()[:, 0:QW])
            rq_sq = rows.tile([1, QW], F32, tag="rqsq")
            nc.scalar.activation(rq_sq, ssq_all,
                                 mybir.ActivationFunctionType.Sqrt,
                                 scale=float(1.0 / D), bias=eps_sb[0:1, :])
            rq_row = rows.tile([1, QW], F32, tag="rqrow")
            nc.vector.reciprocal_approx_fast(rq_row, rq_sq)
            nc.vector.tensor_scalar_mul(rq_row, rq_row, ISD)
            rqrep = rows.tile([128, QW], F32, tag="rqrep")
            nc.gpsimd.partition_broadcast(rqrep, rq_row)
            for h in range(3):
                nc.vector.scalar_tensor_tensor(
                    out=qT[h], in0=qraw[h], scalar=gq_sb[:, h:h + 1], in1=rqrep,
                    op0=mybir.AluOpType.mult, op1=mybir.AluOpType.mult)

            # k segment 1: partition-major gather [128, KCOL1]
            ccr1 = rows.tile([128, KCOL1], F32, tag="ccr1")
            nc.sync.dma_start(out=ccr1, in_=bass.AP(
                tensor=T["cc1_out"].ap().tensor, offset=QW,
                ap=[[1, 128], [128, KCOL1]]))
            ra1 = rows.tile([128, KCOL1], F32, tag="ra1")
            nc.scalar.activation(ra1, ccr1, mybir.ActivationFunctionType.Sqrt,
                                 scale=float(1.0 / D), bias=eps_sb)
            rmsk_a = rows.tile([128, KCOL1], F32, tag="rmska")
            nc.vector.reciprocal(rmsk_a, ra1)

            # k segment 2 (from AR#2): [128, KCOL2]
            ccr2 = rows.tile([128, KCOL2], F32, tag="ccr2")
            nc.sync.dma_start(out=ccr2, in_=bass.AP(
                tensor=T["cc2_out"].ap().tensor, offset=0,
                ap=[[1, 128], [128, KCOL2]]))
            ra2 = rows.tile([128, KCOL2], F32, tag="ra2")
            nc.scalar.activation(ra2, ccr2, mybir.ActivationFunctionType.Sqrt,
                                 scale=float(1.0 / D), bias=eps_sb)
            rmsk_b = rows.tile([128, KCOL2], F32, tag="rmskb")
            nc.vector.reciprocal(rmsk_b, ra2)

        # =============== P3: attention + O projection ===============
        # The ones/PV matmuls wait on the exp (ACT); emit them a few k-tiles
        # late so the in-order PE queue never stalls on the ACT latency.
        with tc.tile_pool(name="pat", bufs=2) as pat, \
             tc.tile_pool(name="pps2", bufs=2, space="PSUM") as pps2, \
             tc.tile_pool(name="ppo", bufs=1, space="PSUM") as ppo, \
             tc.tile_pool(name="ppm", bufs=1, space="PSUM") as ppm:
            LOOK = 3
            wo_sb = keep.tile([128, 3, KC, 128], BF16, tag="wo")
            nc.sync.dma_start(out=wo_sb,
                              in_=T["woT"].ap().rearrange("h c p n -> p h c n"))

            def rmsk_sl(kt, kk):
                if kt < KCOL1:
                    return rmsk_a[:kk, kt:kt + 1]
                return rmsk_b[:kk, kt - KCOL1:kt - KCOL1 + 1]

            for qf in range(NFR):
                krange = FRT * (qf + 1)
                nkt = (krange + 127) // 128
                attn = []
                for h in range(3):
                    ps_o = ppo.tile([128, 2, 512], F32, tag="po")
                    ps_d = ppm.tile([1, 2, 512], F32, tag="pmisc")
                    pending = []      # (kt, kk, p_sb) awaiting PV
                    dpend = []        # full exp tiles awaiting quad-add
                    dgroups = []      # (tile, kk) for the denominator matmuls

                    def flush_one():
                        kt0, kk0, p0 = pending.pop(0)
                        for b in range(2):
                            nc.tensor.matmul(ps_o[:, b, 0:390],
                                             v_sb[kt0][:kk0, h * 128:(h + 1) * 128],
                                             p0[:kk0, b, :],
                                             start=(kt0 == 0),
                                             stop=(kt0 == nkt - 1))

                    def pop_dpend():
                        if len(dpend) >= 2:
                            a1 = pat.tile([128, 2, 390], BF16, tag="padd",
                                          bufs=6)
                            nc.vector.tensor_add(a1, dpend[0], dpend[1])
                            if len(dpend) == 4:
                                a2 = pat.tile([128, 2, 390], BF16, tag="padd", bufs=6)
                                nc.vector.tensor_add(a2, dpend[2], dpend[3])
                                a3 = pat.tile([128, 2, 390], BF16, tag="padd", bufs=6)
                                nc.vector.tensor_add(a3, a1, a2)
                                dgroups.append((a3, 128))
                            elif len(dpend) == 3:
                                a3 = pat.tile([128, 2, 390], BF16, tag="padd", bufs=6)
                                nc.vector.tensor_add(a3, a1, dpend[2])
                                dgroups.append((a3, 128))
                            else:
                                dgroups.append((a1, 128))
                        elif dpend:
                            dgroups.append((dpend[0], 128))
                        dpend.clear()

                    for kt in range(nkt):
                        kk = min(128, krange - kt * 128)
                        ps_s = pps2.tile([128, 2, 512], F32, tag="ps")
                        for b in range(2):
                            nc.tensor.matmul(
                                ps_s[:kk, b, 0:390],
                                kT[h][:, kt * 128:kt * 128 + kk],
                                qT[h][:, qf * QH + b * 390: qf * QH + (b + 1) * 390],
                                start=True, stop=True)
                        p_sb = pat.tile([128, 2, 390], BF16, tag="p", bufs=10)
                        nc.scalar.activation(p_sb[:kk], ps_s[:kk, :, 0:390],
                                             mybir.ActivationFunctionType.Exp,
                                             scale=rmsk_sl(kt, kk))
                        pending.append((kt, kk, p_sb))
                        if kk == 128:
                            dpend.append(p_sb)
                            if len(dpend) == 4:
                                pop_dpend()
                        else:
                            pop_dpend()
                            dgroups.append((p_sb, kk))
                        if len(pending) > LOOK:
                            flush_one()
                    while pending:
                        flush_one()
                    pop_dpend()
                    # denominator matmuls, all deps long satisfied by now
                    for gi, (gt, gk) in enumerate(dgroups):
                        for b in range(2):
                            nc.tensor.matmul(ps_d[:, b, 0:390], ones_sb[:gk, :],
                                             gt[:gk, b, :],
                                             start=(gi == 0),
                                             stop=(gi == len(dgroups) - 1))
                    drecip = pat.tile([1, 2, 390], F32, tag="dr")
                    nc.vector.reciprocal_approx_fast(drecip, ps_d[:, :, 0:390])
                    drep = pat.tile([128, 2, 390], F32, tag="drep")
                    nc.gpsimd.partition_broadcast(
                        drep.rearrange("p a b -> p (a b)"),
                        drecip.rearrange("p a b -> p (a b)"))
                    at = pat.tile([128, 2, 390], BF16, tag=f"at{h}")
                    nc.vector.tensor_mul(at, ps_o[:, :, 0:390], drep)
                    attn.append(at)
                for oc in range(KC):
                    ps_oc = ppm.tile([128, 2, 512], F32, tag="pmisc")
                    for h in range(3):
                        for b in range(2):
                            nc.tensor.matmul(ps_oc[:, b, 0:390], wo_sb[:, h, oc, :],
                                             attn[h][:, b, :],
                                             start=(h == 0), stop=(h == 2))
                    o_sb = pat.tile([128, 2, 390], F16, tag="osb")
                    nc.vector.tensor_copy(o_sb, ps_oc[:, :, 0:390])
                    nc.sync.dma_start(
                        out=T["outOT"].ap()[oc * 128:(oc + 1) * 128,
                                            qf * QH:(qf + 1) * QH],
                        in_=o_sb.rearrange("p a b -> p (a b)"))


def _prep(inputs):
    g = {k: np.asarray(v) for k, v in inputs.items()}
    x = g["x"].astype(np.float32)
    assert x.shape == (1, L, D), x.shape
    has_bias = any(np.any(g[k].astype(np.float64)) for k in ("bq", "bk", "bv"))
    has_gk = not np.all(g["gk"] == 1.0)

    perm = np.empty(D, np.int64)
    for h in range(NH):
        base = h * HD
        perm[base:base + 64] = base + 2 * np.arange(64)
        perm[base + 64:base + 128] = base + 2 * np.arange(64) + 1

    cos_t, sin_t = _rope_tables(g["freqs"].astype(np.float32))  # [64, L]

    xT = np.ascontiguousarray(x[0].T).astype(BF)                # [D, L]
    xT_dev = np.ascontiguousarray(xT.reshape(KC, 128, L))

    wq_p = g["wq"].astype(np.float32)[perm]   # q feature j = row j of wq
    wk_p = g["wk"].astype(np.float32)[perm]
    gq_p = g["gq"].astype(np.float32)[perm]
    gk_p = g["gk"].astype(np.float32)[perm]

    cos_bf = cos_t.astype(BF)
    sin_bf = sin_t.astype(BF)

    in_maps = []
    qcols_all = []
    for c in range(8):
        gidx, s = c % 4, c // 4
        heads = [3 * gidx, 3 * gidx + 1, 3 * gidx + 2]
        cols = np.concatenate([np.arange(h * HD, (h + 1) * HD) for h in heads])
        qcols = np.concatenate(
            [np.arange(f * FRT + s * QH, f * FRT + (s + 1) * QH)
             for f in range(NFR)])
        qcols_all.append(qcols)

        woT = np.empty((3, KC, 128, 128), np.float32)
        for hl, h in enumerate(heads):
            for oc in range(KC):
                woT[hl, oc] = g["wo"].astype(np.float32)[
                    oc * 128:(oc + 1) * 128, h * HD:(h + 1) * HD].T
        m = dict(
            xT=xT_dev,
            xTq=np.ascontiguousarray(xT_dev[:, :, qcols]),
            wqT=np.ascontiguousarray(wq_p[cols].T).astype(BF).reshape(KC, 128, 384),
            wkT=np.ascontiguousarray(wk_p[cols].T).astype(BF).reshape(KC, 128, 384),
            wvT=np.ascontiguousarray(
                g["wv"].astype(np.float32)[cols].T).astype(BF).reshape(KC, 128, 384),
            woT=woT.astype(BF),
            cosk=cos_bf, sink=sin_bf,
            cosq=np.ascontiguousarray(cos_bf[:, qcols]),
            sinq=np.ascontiguousarray(sin_bf[:, qcols]),
            gq_d=np.ascontiguousarray(gq_p[cols].reshape(3, 128).T),
        )
        if has_gk:
            m["gk_d"] = np.ascontiguousarray(gk_p[cols].reshape(3, 128).T)
        if has_bias:
            bq_p = g["bq"].astype(np.float32)[perm]
            bk_p = g["bk"].astype(np.float32)[perm]
            m["bqkv"] = np.stack(
                [bq_p[cols], bk_p[cols], g["bv"].astype(np.float32)[cols]]
            ).reshape(1, 3, 384).astype(BF)
        m = {k: np.ascontiguousarray(v) for k, v in m.items()}
        in_maps.append(m)
    return in_maps, qcols_all, (has_bias, has_gk), g


def kernel(**inputs):
    in_maps, qcols_all, flags, g = _prep(inputs)
    if flags not in _nc_cache:
        _nc_cache[flags] = _build(*flags)
    nc = _nc_cache[flags]
    res = bass_utils.run_bass_kernel_spmd(nc, in_maps, core_ids=list(range(8)))
    out = np.zeros((D, L), np.float32)
    for c in range(8):
        out[:, qcols_all[c]] += res.results[c]["outOT"].astype(np.float32)
    out = out.T[None, :, :] + g["bo"].astype(np.float32)
    return out.astype(np.float32)


# revision 10
# speedup vs baseline: 1.0119x; 1.0119x over previous
"""CausalWanSelfAttention on 8 trn2 NeuronCores (Bass/Tile, SPMD).

Sharding: 4 head-groups (3 heads each) x 2 query-halves (780 q-rows per frame).
The frame mask (F=3, sink=1, local=2) is exactly frame-block-causal, so
attention is dense per (qframe, kframe<=qframe) block.

v2 pipeline per device:
  One pass over x computes K (feature-major, rope fused into PSUM evac) AND
  V (token-major) from shared x tiles; Q proj (core's 2340 tokens) likewise
  rope-fused.  Sum-of-squares partials accumulate in SBUF row tiles and go
  out as TWO mask-free AllReduces over the same-query-half 4-core groups
  ({0,2,4,6}/{1,3,5,7}): AR#1 = [ss_q | ss_k(0:3456)] issued before the K/V
  tail (tokens 3456:4680) so the tail + frame-0/1 attention hide AR latency;
  AR#2 = ss_k(3456:) hidden under frame-0/1 attention.  rms_k folds into the
  exp scale (per-partition in transposed-score layout), rms_q (and 1/sqrt(d))
  folds into qT.  Transposed-score flash attention without max-subtraction;
  softmax denominators via ones-column matmuls over quad-added exp tiles;
  per-core partial O-projection (fp16) summed on the host.
"""
import sys
sys.path.insert(0, '/opt/trn_rl_repo')

import numpy as np
import ml_dtypes

import concourse.bass as bass
import concourse.mybir as mybir
import concourse.tile as tile
from concourse import bacc, bass_utils

F32 = mybir.dt.float32
F16 = mybir.dt.float16
BF16 = mybir.dt.bfloat16
BF = ml_dtypes.bfloat16

L, D, NH, HD = 4680, 1536, 12, 128
NFR, FRT = 3, 1560          # frames, tokens per frame
QH = 780                    # q rows per (core, frame)
QW = 3 * QH                 # 2340 q tokens per core
KC = 12                     # contraction chunks of 128
TG = 384                    # token group for k/v proj (12 full + 72 runt)
QTG = 468                   # token group for q proj (5 exact)
KSPLIT = 3456               # k tokens covered by AR#1 (groups 0..8)
KCOL1 = KSPLIT // 128       # 27
LPAD = 4736                 # 37*128
KCOL2 = (LPAD - KSPLIT) // 128  # 10 (last col has 72 valid)
CC1 = QW + KSPLIT           # 5796 f32
CC2 = LPAD - KSPLIT         # 1280 f32 (tail zero-padded)
EPS = 1e-6
ISD = float(1.0 / np.sqrt(HD))

_nc_cache = {}


def _rope_tables(freqs):
    """cos/sin [64, L] float32 per rope pair, matching reference _rope_table."""
    c = freqs.shape[1]           # 64
    s0 = c - 2 * (c // 3)        # 22
    s1 = c // 3                  # 21
    Fr, H, W = NFR, 30, 52
    fr = np.zeros((Fr, H, W, c, 2), np.float32)
    fr[:, :, :, :s0] = freqs[:Fr, :s0].reshape(Fr, 1, 1, s0, 2)
    fr[:, :, :, s0:s0 + s1] = freqs[:H, s0:s0 + s1].reshape(1, H, 1, s1, 2)
    fr[:, :, :, s0 + s1:] = freqs[:W, s0 + s1:].reshape(1, 1, W, s1, 2)
    fr = fr.reshape(L, c, 2)
    return fr[:, :, 0].T.copy(), fr[:, :, 1].T.copy()  # [64, L] each


def _build(has_bias, has_gk):
    nc = bacc.Bacc(trn_type="TRN2", debug=False, num_devices=8)
    ExtIn = dict(kind="ExternalInput")
    T = {}
    T["xT"] = nc.dram_tensor("xT", [KC, 128, L], BF16, **ExtIn)
    T["xTq"] = nc.dram_tensor("xTq", [KC, 128, QW], BF16, **ExtIn)
    T["wqT"] = nc.dram_tensor("wqT", [KC, 128, 384], BF16, **ExtIn)
    T["wkT"] = nc.dram_tensor("wkT", [KC, 128, 384], BF16, **ExtIn)
    T["wvT"] = nc.dram_tensor("wvT", [KC, 128, 384], BF16, **ExtIn)
    T["woT"] = nc.dram_tensor("woT", [3, KC, 128, 128], BF16, **ExtIn)
    T["cosk"] = nc.dram_tensor("cosk", [64, L], BF16, **ExtIn)
    T["sink"] = nc.dram_tensor("sink", [64, L], BF16, **ExtIn)
    T["cosq"] = nc.dram_tensor("cosq", [64, QW], BF16, **ExtIn)
    T["sinq"] = nc.dram_tensor("sinq", [64, QW], BF16, **ExtIn)
    T["gq_d"] = nc.dram_tensor("gq_d", [128, 3], F32, **ExtIn)
    if has_gk:
        T["gk_d"] = nc.dram_tensor("gk_d", [128, 3], F32, **ExtIn)
    if has_bias:
        T["bqkv"] = nc.dram_tensor("bqkv", [1, 3, 384], BF16, **ExtIn)
    T["outOT"] = nc.dram_tensor("outOT", [D, QW], F16, kind="ExternalOutput")
    T["ccq_in"] = nc.dram_tensor("ccq_in", [1, QW], F32)
    T["ccq_out"] = nc.dram_tensor("ccq_out", [1, QW], F32)
    T["cc1_in"] = nc.dram_tensor("cc1_in", [1, KSPLIT], F32)
    T["cc1_out"] = nc.dram_tensor("cc1_out", [1, KSPLIT], F32)
    T["cc2_in"] = nc.dram_tensor("cc2_in", [1, CC2], F32)
    T["cc2_out"] = nc.dram_tensor("cc2_out", [1, CC2], F32)

    with tile.TileContext(nc) as tc:
        _emit(nc, tc, T, has_bias, has_gk)
    nc.compile()
    return nc


def _emit(nc, tc, T, has_bias, has_gk):
    from contextlib import ExitStack
    RG = [[0, 1, 2, 3], [4, 5, 6, 7]]
    es = ExitStack()
    with es:
        keep = es.enter_context(tc.tile_pool(name="keep", bufs=1))
        rows = es.enter_context(tc.tile_pool(name="rows", bufs=1))

        gq_sb = keep.tile([128, 3], F32, tag="gq")
        nc.sync.dma_start(out=gq_sb, in_=T["gq_d"].ap())
        if has_gk:
            gk_sb = keep.tile([128, 3], F32, tag="gk")
            nc.sync.dma_start(out=gk_sb, in_=T["gk_d"].ap())
        ones_sb = keep.tile([128, 1], BF16, tag="ones")
        nc.vector.memset(ones_sb, 1.0)
        eps_sb = keep.tile([128, 1], F32, tag="eps")
        nc.vector.memset(eps_sb, EPS)
        if has_bias:
            b_sb = keep.tile([1, 3, 384], BF16, tag="bqkv")
            nc.sync.dma_start(out=b_sb, in_=T["bqkv"].ap())
            onesrow = keep.tile([1, 512], BF16, tag="onesrow")
            nc.vector.memset(onesrow, 1.0)

        kT = [keep.tile([128, L], BF16, tag=f"kT{h}", name=f"kT{h}")
              for h in range(3)]
        qraw = [keep.tile([128, QW], BF16, tag=f"qraw{h}", name=f"qraw{h}")
                for h in range(3)]
        qT = [keep.tile([128, QW], BF16, tag=f"qT{h}", name=f"qT{h}")
              for h in range(3)]
        ntok_tiles = (L + 127) // 128  # 37
        v_sb = [keep.tile([min(128, L - i * 128), 384], BF16, tag=f"v{i}",
                          name=f"v{i}")
                for i in range(ntok_tiles)]


        # =============== P1: projections + rope + ss partials ===============
        with tc.tile_pool(name="pw", bufs=1) as pw, \
             tc.tile_pool(name="pxk", bufs=2) as pxk, \
             tc.tile_pool(name="ptab", bufs=2) as ptab, \
             tc.tile_pool(name="tmp", bufs=1) as tmp, \
             tc.tile_pool(name="tmp2", bufs=2) as tmp2, \
             tc.tile_pool(name="ppk", bufs=2, space="PSUM") as ppk, \
             tc.tile_pool(name="ppv", bufs=2, space="PSUM") as ppv, \
             tc.tile_pool(name="pps", bufs=2, space="PSUM") as pps:
            wq_c, wk_c, wv_c = [], [], []
            for kc in range(KC):
                t = pw.tile([128, 384], BF16, tag=f"wq{kc}", name=f"wq{kc}")
                nc.sync.dma_start(out=t, in_=T["wqT"].ap()[kc])
                wq_c.append(t)
            for kc in range(KC):
                t = pw.tile([128, 384], BF16, tag=f"wk{kc}", name=f"wk{kc}")
                nc.sync.dma_start(out=t, in_=T["wkT"].ap()[kc])
                wk_c.append(t)
                t = pw.tile([128, 384], BF16, tag=f"wv{kc}", name=f"wv{kc}")
                nc.sync.dma_start(out=t, in_=T["wvT"].ap()[kc])
                wv_c.append(t)

            def rope_evac(psum, cos_sl, sin_sl, dst, col0, n, g_sl):
                t1 = tmp.tile([64, QTG], F32, tag="t1")
                t2 = tmp.tile([64, QTG], F32, tag="t2")
                t3 = tmp.tile([64, QTG], F32, tag="t3")
                t4 = tmp.tile([64, QTG], F32, tag="t4")
                nc.vector.tensor_mul(t1[:, :n], psum[0:64, :n], cos_sl)
                nc.vector.tensor_mul(t2[:, :n], psum[64:128, :n], sin_sl)
                nc.vector.tensor_mul(t3[:, :n], psum[0:64, :n], sin_sl)
                nc.vector.tensor_mul(t4[:, :n], psum[64:128, :n], cos_sl)
                nc.vector.tensor_sub(dst[0:64, col0:col0 + n], t1[:, :n], t2[:, :n])
                nc.vector.tensor_add(dst[64:128, col0:col0 + n], t3[:, :n], t4[:, :n])
                # ss partial from the roped (pre-gain) values
                sq = tmp2.tile([128, QTG], BF16, tag="sq", bufs=8)
                nc.scalar.square(sq[:, :n], dst[:, col0:col0 + n])
                if g_sl is not None:
                    gtmp = tmp2.tile([128, QTG], BF16, tag="gtmp")
                    nc.vector.tensor_scalar_mul(gtmp[:, :n],
                                                dst[:, col0:col0 + n], g_sl)
                    nc.scalar.copy(out=dst[:, col0:col0 + n], in_=gtmp[:, :n])
                return sq

            # Deferred ss emission: the ones-matmul waits on Square(ACT); by
            # lagging one token-group the PE queue never stalls on it.
            ss_pending = []

            def flush_ss():
                for sqs, ss_ps, n, dram, off in ss_pending:
                    for h, sq in enumerate(sqs):
                        nc.tensor.matmul(ss_ps[:, :n], ones_sb, sq[:, :n],
                                         start=(h == 0), stop=(h == 2))
                    st = tmp2.tile([1, QTG], F32, tag="ssst")
                    nc.vector.tensor_copy(st[:, :n], ss_ps[:, :n])
                    nc.sync.dma_start(out=dram.ap()[:, off:off + n],
                                      in_=st[:, :n])
                ss_pending.clear()

            def kv_group(tg):
                c0 = tg * TG
                n = min(TG, L - c0)
                xk = pxk.tile([128, KC, QTG], BF16, tag="xk")
                nc.sync.dma_start(
                    out=xk[:, :, :n],
                    in_=T["xT"].ap()[:, :, c0:c0 + n].rearrange("c p n -> p c n"))
                ck = ptab.tile([64, QTG], BF16, tag="cs")
                nc.sync.dma_start(out=ck[:, :n], in_=T["cosk"].ap()[:, c0:c0 + n])
                sk = ptab.tile([64, QTG], BF16, tag="sn")
                nc.sync.dma_start(out=sk[:, :n], in_=T["sink"].ap()[:, c0:c0 + n])
                ss_ps = pps.tile([1, QTG], F32, tag="pss")
                sqs = []
                for h in range(3):
                    psk = ppk.tile([128, QTG], F32, tag="pk", bufs=3)
                    for kc in range(KC):
                        nc.tensor.matmul(psk[:, :n],
                                         wk_c[kc][:, h * 128:(h + 1) * 128],
                                         xk[:, kc, :n], start=(kc == 0),
                                         stop=(not has_bias and kc == KC - 1))
                    if has_bias:
                        nc.tensor.matmul(psk[:, :n],
                                         b_sb[:, 1, h * 128:(h + 1) * 128],
                                         onesrow[:, :n], start=False, stop=True)
                    g_sl = gk_sb[:, h:h + 1] if has_gk else None
                    sqs.append(rope_evac(psk, ck[:, :n], sk[:, :n], kT[h], c0, n,
                                         g_sl))
                # V proj from the same x tiles (token-major)
                for j in range(3):
                    vi = tg * 3 + j
                    if vi >= ntok_tiles or vi * 128 >= c0 + n:
                        break
                    rsz = v_sb[vi].shape[0]
                    j0 = vi * 128 - c0
                    psv = ppv.tile([128, 384], F32, tag="pv")
                    for kc in range(KC):
                        nc.tensor.matmul(psv[:rsz, :], xk[:, kc, j0:j0 + rsz],
                                         wv_c[kc], start=(kc == 0),
                                         stop=(not has_bias and kc == KC - 1))
                    if has_bias:
                        nc.tensor.matmul(psv[:rsz, :], onesrow[:, :rsz],
                                         b_sb[:, 2, :], start=False, stop=True)
                    nc.vector.tensor_copy(v_sb[vi], psv[:rsz, :])
                flush_ss()
                if c0 < KSPLIT:
                    ss_pending.append((sqs, ss_ps, n, T["cc1_in"], c0))
                else:
                    ss_pending.append((sqs, ss_ps, n, T["cc2_in"], c0 - KSPLIT))

            ng = (L + TG - 1) // TG  # 13
            ng1 = KSPLIT // TG       # 9 groups before AR-k1

            # --- Q projection first (core's 2340 tokens): 5 groups of 468
            for tg in range(QW // QTG):
                c0 = tg * QTG
                n = QTG
                xq = pxk.tile([128, KC, QTG], BF16, tag="xk")
                nc.sync.dma_start(
                    out=xq,
                    in_=T["xTq"].ap()[:, :, c0:c0 + n].rearrange("c p n -> p c n"))
                cq = ptab.tile([64, QTG], BF16, tag="cs")
                nc.sync.dma_start(out=cq, in_=T["cosq"].ap()[:, c0:c0 + n])
                sq_t = ptab.tile([64, QTG], BF16, tag="sn")
                nc.sync.dma_start(out=sq_t, in_=T["sinq"].ap()[:, c0:c0 + n])
                ss_ps = pps.tile([1, QTG], F32, tag="pss")
                sqs = []
                for h in range(3):
                    psq = ppk.tile([128, QTG], F32, tag="pk", bufs=3)
                    for kc in range(KC):
                        nc.tensor.matmul(psq,
                                         wq_c[kc][:, h * 128:(h + 1) * 128],
                                         xq[:, kc, :], start=(kc == 0),
                                         stop=(not has_bias and kc == KC - 1))
                    if has_bias:
                        nc.tensor.matmul(psq, b_sb[:, 0, h * 128:(h + 1) * 128],
                                         onesrow[:, :n], start=False, stop=True)
                    sqs.append(rope_evac(psq, cq, sq_t, qraw[h], c0, n, None))
                flush_ss()
                ss_pending.append((sqs, ss_ps, n, T["ccq_in"], c0))
            flush_ss()

            # =============== AR-q ===========================================
            nc.gpsimd.collective_compute(
                "AllReduce", mybir.AluOpType.add, replica_groups=RG,
                ins=[T["ccq_in"].ap().opt()], outs=[T["ccq_out"].ap().opt()])

            # rms-q chain (hides under the K/V pass below)
            ssq_all = rows.tile([1, QW], F32, tag="ssqall")
            nc.sync.dma_start(out=ssq_all, in_=T["ccq_out"].ap())
            rq_sq = rows.tile([1, QW], F32, tag="rqsq")
            nc.scalar.activation(rq_sq, ssq_all,
                                 mybir.ActivationFunctionType.Sqrt,
                                 scale=float(1.0 / D), bias=eps_sb[0:1, :])
            rq_row = rows.tile([1, QW], F32, tag="rqrow")
            nc.vector.reciprocal_approx_fast(rq_row, rq_sq)
            nc.vector.tensor_scalar_mul(rq_row, rq_row, ISD)
            rqrep = rows.tile([128, QW], F32, tag="rqrep")
            nc.gpsimd.partition_broadcast(rqrep, rq_row)
            for h in range(3):
                nc.vector.scalar_tensor_tensor(
                    out=qT[h], in0=qraw[h], scalar=gq_sb[:, h:h + 1], in1=rqrep,
                    op0=mybir.AluOpType.mult, op1=mybir.AluOpType.mult)

            # --- K/V head groups 0..8
            for tg in range(ng1):
                kv_group(tg)
            flush_ss()

            # =============== AR-k1: ss_k[0:KSPLIT] ==========================
            nc.gpsimd.collective_compute(
                "AllReduce", mybir.AluOpType.add, replica_groups=RG,
                ins=[T["cc1_in"].ap().opt()], outs=[T["cc1_out"].ap().opt()])

            # --- K/V tail (tokens KSPLIT:L) overlaps AR-k1
            for tg in range(ng1, ng):
                kv_group(tg)
            flush_ss()

            # =============== AR-k2: ss_k[KSPLIT:] ===========================
            zr = tmp2.tile([1, 64], F32, tag="zr")
            nc.vector.memset(zr, 0.0)
            nc.sync.dma_start(out=T["cc2_in"].ap()[:, L - KSPLIT:CC2],
                              in_=zr[:, :CC2 - (L - KSPLIT)])
            nc.gpsimd.collective_compute(
                "AllReduce", mybir.AluOpType.add, replica_groups=RG,
                ins=[T["cc2_in"].ap().opt()], outs=[T["cc2_out"].ap().opt()])

            # k segment 1: partition-major gather [128, KCOL1]
            ccr1 = rows.tile([128, KCOL1], F32, tag="ccr1")
            nc.sync.dma_start(out=ccr1, in_=bass.AP(
                tensor=T["cc1_out"].ap().tensor, offset=0,
                ap=[[1, 128], [128, KCOL1]]))
            ra1 = rows.tile([128, KCOL1], F32, tag="ra1")
            nc.scalar.activation(ra1, ccr1, mybir.ActivationFunctionType.Sqrt,
                                 scale=float(1.0 / D), bias=eps_sb)
            rmsk_a = rows.tile([128, KCOL1], F32, tag="rmska")
            nc.vector.reciprocal(rmsk_a, ra1)

            # k segment 2 (from AR#2): [128, KCOL2]
            ccr2 = rows.tile([128, KCOL2], F32, tag="ccr2")
            nc.sync.dma_start(out=ccr2, in_=bass.AP(
                tensor=T["cc2_out"].ap().tensor, offset=0,
                ap=[[1, 128], [128, KCOL2]]))
            ra2 = rows.tile([128, KCOL2], F32, tag="ra2")
            nc.scalar.activation(ra2, ccr2, mybir.ActivationFunctionType.Sqrt,
                                 scale=float(1.0 / D), bias=eps_sb)
            rmsk_b = rows.tile([128, KCOL2], F32, tag="rmskb")
            nc.vector.reciprocal(rmsk_b, ra2)

        # =============== P3: attention + O projection ===============
        # The ones/PV matmuls wait on the exp (ACT); emit them a few k-tiles
        # late so the in-order PE queue never stalls on the ACT latency.
        with tc.tile_pool(name="pat", bufs=2) as pat, \
             tc.tile_pool(name="pps2", bufs=2, space="PSUM") as pps2, \
             tc.tile_pool(name="ppo", bufs=1, space="PSUM") as ppo, \
             tc.tile_pool(name="ppm", bufs=1, space="PSUM") as ppm:
            LOOK = 3
            wo_sb = keep.tile([128, 3, KC, 128], BF16, tag="wo")
            nc.sync.dma_start(out=wo_sb,
                              in_=T["woT"].ap().rearrange("h c p n -> p h c n"))

            def rmsk_sl(kt, kk):
                if kt < KCOL1:
                    return rmsk_a[:kk, kt:kt + 1]
                return rmsk_b[:kk, kt - KCOL1:kt - KCOL1 + 1]

            for qf in range(NFR):
                krange = FRT * (qf + 1)
                nkt = (krange + 127) // 128
                attn = []
                for h in range(3):
                    ps_o = ppo.tile([128, 2, 512], F32, tag="po")
                    ps_d = ppm.tile([1, 2, 512], F32, tag="pmisc")
                    pending = []      # (kt, kk, p_sb) awaiting PV
                    dpend = []        # full exp tiles awaiting quad-add
                    dgroups = []      # (tile, kk) for the denominator matmuls

                    def flush_one():
                        kt0, kk0, p0 = pending.pop(0)
                        for b in range(2):
                            nc.tensor.matmul(ps_o[:, b, 0:390],
                                             v_sb[kt0][:kk0, h * 128:(h + 1) * 128],
                                             p0[:kk0, b, :],
                                             start=(kt0 == 0),
                                             stop=(kt0 == nkt - 1))

                    def pop_dpend():
                        if len(dpend) >= 2:
                            a1 = pat.tile([128, 2, 390], BF16, tag="padd",
                                          bufs=6)
                            nc.vector.tensor_add(a1, dpend[0], dpend[1])
                            if len(dpend) == 4:
                                a2 = pat.tile([128, 2, 390], BF16, tag="padd", bufs=6)
                                nc.vector.tensor_add(a2, dpend[2], dpend[3])
                                a3 = pat.tile([128, 2, 390], BF16, tag="padd", bufs=6)
                                nc.vector.tensor_add(a3, a1, a2)
                                dgroups.append((a3, 128))
                            elif len(dpend) == 3:
                                a3 = pat.tile([128, 2, 390], BF16, tag="padd", bufs=6)
                                nc.vector.tensor_add(a3, a1, dpend[2])
                                dgroups.append((a3, 128))
                            else:
                                dgroups.append((a1, 128))
                        elif dpend:
                            dgroups.append((dpend[0], 128))
                        dpend.clear()

                    for kt in range(nkt):
                        kk = min(128, krange - kt * 128)
                        ps_s = pps2.tile([128, 2, 512], F32, tag="ps")
                        for b in range(2):
                            nc.tensor.matmul(
                                ps_s[:kk, b, 0:390],
                                kT[h][:, kt * 128:kt * 128 + kk],
                                qT[h][:, qf * QH + b * 390: qf * QH + (b + 1) * 390],
                                start=True, stop=True)
                        p_sb = pat.tile([128, 2, 390], BF16, tag="p", bufs=10)
                        nc.scalar.activation(p_sb[:kk], ps_s[:kk, :, 0:390],
                                             mybir.ActivationFunctionType.Exp,
                                             scale=rmsk_sl(kt, kk))
                        pending.append((kt, kk, p_sb))
                        if kk == 128:
                            dpend.append(p_sb)
                            if len(dpend) == 4:
                                pop_dpend()
                        else:
                            pop_dpend()
                            dgroups.append((p_sb, kk))
                        if len(pending) > LOOK:
                            flush_one()
                    while pending:
                        flush_one()
                    pop_dpend()
                    # denominator matmuls, all deps long satisfied by now
                    for gi, (gt, gk) in enumerate(dgroups):
                        for b in range(2):
                            nc.tensor.matmul(ps_d[:, b, 0:390], ones_sb[:gk, :],
                                             gt[:gk, b, :],
                                             start=(gi == 0),
                                             stop=(gi == len(dgroups) - 1))
                    drecip = pat.tile([1, 2, 390], F32, tag="dr")
                    nc.vector.reciprocal_approx_fast(drecip, ps_d[:, :, 0:390])
                    drep = pat.tile([128, 2, 390], F32, tag="drep")
                    nc.gpsimd.partition_broadcast(
                        drep.rearrange("p a b -> p (a b)"),
                        drecip.rearrange("p a b -> p (a b)"))
                    at = pat.tile([128, 2, 390], BF16, tag=f"at{h}")
                    nc.vector.tensor_mul(at, ps_o[:, :, 0:390], drep)
                    attn.append(at)
                for oc in range(KC):
                    ps_oc = ppm.tile([128, 2, 512], F32, tag="pmisc")
                    for h in range(3):
                        for b in range(2):
                            nc.tensor.matmul(ps_oc[:, b, 0:390], wo_sb[:, h, oc, :],
                                             attn[h][:, b, :],
                                             start=(h == 0), stop=(h == 2))
                    o_sb = pat.tile([128, 2, 390], F16, tag="osb")
                    nc.vector.tensor_copy(o_sb, ps_oc[:, :, 0:390])
                    nc.sync.dma_start(
                        out=T["outOT"].ap()[oc * 128:(oc + 1) * 128,
                                            qf * QH:(qf + 1) * QH],
                        in_=o_sb.rearrange("p a b -> p (a b)"))


def _prep(inputs):
    g = {k: np.asarray(v) for k, v in inputs.items()}
    x = g["x"].astype(np.float32)
    assert x.shape == (1, L, D), x.shape
    has_bias = any(np.any(g[k].astype(np.float64)) for k in ("bq", "bk", "bv"))
    has_gk = not np.all(g["gk"] == 1.0)

    perm = np.empty(D, np.int64)
    for h in range(NH):
        base = h * HD
        perm[base:base + 64] = base + 2 * np.arange(64)
        perm[base + 64:base + 128] = base + 2 * np.arange(64) + 1

    cos_t, sin_t = _rope_tables(g["freqs"].astype(np.float32))  # [64, L]

    xT = np.ascontiguousarray(x[0].T).astype(BF)                # [D, L]
    xT_dev = np.ascontiguousarray(xT.reshape(KC, 128, L))

    wq_p = g["wq"].astype(np.float32)[perm]   # q feature j = row j of wq
    wk_p = g["wk"].astype(np.float32)[perm]
    gq_p = g["gq"].astype(np.float32)[perm]
    gk_p = g["gk"].astype(np.float32)[perm]

    cos_bf = cos_t.astype(BF)
    sin_bf = sin_t.astype(BF)

    in_maps = []
    qcols_all = []
    for c in range(8):
        gidx, s = c % 4, c // 4
        heads = [3 * gidx, 3 * gidx + 1, 3 * gidx + 2]
        cols = np.concatenate([np.arange(h * HD, (h + 1) * HD) for h in heads])
        qcols = np.concatenate(
            [np.arange(f * FRT + s * QH, f * FRT + (s + 1) * QH)
             for f in range(NFR)])
        qcols_all.append(qcols)

        woT = np.empty((3, KC, 128, 128), np.float32)
        for hl, h in enumerate(heads):
            for oc in range(KC):
                woT[hl, oc] = g["wo"].astype(np.float32)[
                    oc * 128:(oc + 1) * 128, h * HD:(h + 1) * HD].T
        m = dict(
            xT=xT_dev,
            xTq=np.ascontiguousarray(xT_dev[:, :, qcols]),
            wqT=np.ascontiguousarray(wq_p[cols].T).astype(BF).reshape(KC, 128, 384),
            wkT=np.ascontiguousarray(wk_p[cols].T).astype(BF).reshape(KC, 128, 384),
            wvT=np.ascontiguousarray(
                g["wv"].astype(np.float32)[cols].T).astype(BF).reshape(KC, 128, 384),
            woT=woT.astype(BF),
            cosk=cos_bf, sink=sin_bf,
            cosq=np.ascontiguousarray(cos_bf[:, qcols]),
            sinq=np.ascontiguousarray(sin_bf[:, qcols]),
            gq_d=np.ascontiguousarray(gq_p[cols].reshape(3, 128).T),
        )
        if has_gk:
            m["gk_d"] = np.ascontiguousarray(gk_p[cols].reshape(3, 128).T)
        if has_bias:
            bq_p = g["bq"].astype(np.float32)[perm]
            bk_p = g["bk"].astype(np.float32)[perm]
            m["bqkv"] = np.stack(
                [bq_p[cols], bk_p[cols], g["bv"].astype(np.float32)[cols]]
            ).reshape(1, 3, 384).astype(BF)
        m = {k: np.ascontiguousarray(v) for k, v in m.items()}
        in_maps.append(m)
    return in_maps, qcols_all, (has_bias, has_gk), g


def kernel(**inputs):
    in_maps, qcols_all, flags, g = _prep(inputs)
    if flags not in _nc_cache:
        _nc_cache[flags] = _build(*flags)
    nc = _nc_cache[flags]
    res = bass_utils.run_bass_kernel_spmd(nc, in_maps, core_ids=list(range(8)))
    out = np.zeros((D, L), np.float32)
    for c in range(8):
        out[:, qcols_all[c]] += res.results[c]["outOT"].astype(np.float32)
    out = out.T[None, :, :] + g["bo"].astype(np.float32)
    return out.astype(np.float32)


# revision 11
# speedup vs baseline: 1.0354x; 1.0232x over previous
"""CausalWanSelfAttention on 8 trn2 NeuronCores (Bass/Tile, SPMD).

Sharding: 4 head-groups (3 heads each) x 2 query-halves (780 q-rows per frame).
The frame mask (F=3, sink=1, local=2) is exactly frame-block-causal, so
attention is dense per (qframe, kframe<=qframe) block.

v2 pipeline per device:
  One pass over x computes K (feature-major, rope fused into PSUM evac) AND
  V (token-major) from shared x tiles; Q proj (core's 2340 tokens) likewise
  rope-fused.  Sum-of-squares partials accumulate in SBUF row tiles and go
  out as TWO mask-free AllReduces over the same-query-half 4-core groups
  ({0,2,4,6}/{1,3,5,7}): AR#1 = [ss_q | ss_k(0:3456)] issued before the K/V
  tail (tokens 3456:4680) so the tail + frame-0/1 attention hide AR latency;
  AR#2 = ss_k(3456:) hidden under frame-0/1 attention.  rms_k folds into the
  exp scale (per-partition in transposed-score layout), rms_q (and 1/sqrt(d))
  folds into qT.  Transposed-score flash attention without max-subtraction;
  softmax denominators via ones-column matmuls over quad-added exp tiles;
  per-core partial O-projection (fp16) summed on the host.
"""
import sys
sys.path.insert(0, '/opt/trn_rl_repo')

import numpy as np
import ml_dtypes

import concourse.bass as bass
import concourse.mybir as mybir
import concourse.tile as tile
from concourse import bacc, bass_utils

F32 = mybir.dt.float32
F16 = mybir.dt.float16
BF16 = mybir.dt.bfloat16
BF = ml_dtypes.bfloat16

L, D, NH, HD = 4680, 1536, 12, 128
NFR, FRT = 3, 1560          # frames, tokens per frame
QH = 780                    # q rows per (core, frame)
QW = 3 * QH                 # 2340 q tokens per core
KC = 12                     # contraction chunks of 128
TG = 384                    # token group for k/v proj (12 full + 72 runt)
QTG = 468                   # token group for q proj (5 exact)
KSPLIT = 3456               # k tokens covered by AR#1 (groups 0..8)
KCOL1 = KSPLIT // 128       # 27
LPAD = 4736                 # 37*128
KCOL2 = (LPAD - KSPLIT) // 128  # 10 (last col has 72 valid)
CC1 = QW + KSPLIT           # 5796 f32
CC2 = LPAD - KSPLIT         # 1280 f32 (tail zero-padded)
EPS = 1e-6
ISD = float(1.0 / np.sqrt(HD))

_nc_cache = {}


def _rope_tables(freqs):
    """cos/sin [64, L] float32 per rope pair, matching reference _rope_table."""
    c = freqs.shape[1]           # 64
    s0 = c - 2 * (c // 3)        # 22
    s1 = c // 3                  # 21
    Fr, H, W = NFR, 30, 52
    fr = np.zeros((Fr, H, W, c, 2), np.float32)
    fr[:, :, :, :s0] = freqs[:Fr, :s0].reshape(Fr, 1, 1, s0, 2)
    fr[:, :, :, s0:s0 + s1] = freqs[:H, s0:s0 + s1].reshape(1, H, 1, s1, 2)
    fr[:, :, :, s0 + s1:] = freqs[:W, s0 + s1:].reshape(1, 1, W, s1, 2)
    fr = fr.reshape(L, c, 2)
    return fr[:, :, 0].T.copy(), fr[:, :, 1].T.copy()  # [64, L] each


def _build(has_bias, has_gk):
    nc = bacc.Bacc(trn_type="TRN2", debug=False, num_devices=8)
    ExtIn = dict(kind="ExternalInput")
    T = {}
    T["xT"] = nc.dram_tensor("xT", [KC, 128, L], BF16, **ExtIn)
    T["xTq"] = nc.dram_tensor("xTq", [KC, 128, QW], BF16, **ExtIn)
    T["wqT"] = nc.dram_tensor("wqT", [KC, 128, 384], BF16, **ExtIn)
    T["wkT"] = nc.dram_tensor("wkT", [KC, 128, 384], BF16, **ExtIn)
    T["wvT"] = nc.dram_tensor("wvT", [KC, 128, 384], BF16, **ExtIn)
    T["woT"] = nc.dram_tensor("woT", [3, KC, 128, 128], BF16, **ExtIn)
    T["cosk"] = nc.dram_tensor("cosk", [64, L], BF16, **ExtIn)
    T["sink"] = nc.dram_tensor("sink", [64, L], BF16, **ExtIn)
    T["cosq"] = nc.dram_tensor("cosq", [64, QW], BF16, **ExtIn)
    T["sinq"] = nc.dram_tensor("sinq", [64, QW], BF16, **ExtIn)
    T["gq_d"] = nc.dram_tensor("gq_d", [128, 3], F32, **ExtIn)
    if has_gk:
        T["gk_d"] = nc.dram_tensor("gk_d", [128, 3], F32, **ExtIn)
    if has_bias:
        T["bqkv"] = nc.dram_tensor("bqkv", [1, 3, 384], BF16, **ExtIn)
    T["outOT"] = nc.dram_tensor("outOT", [D, QW], F16, kind="ExternalOutput")
    T["ccq_in"] = nc.dram_tensor("ccq_in", [1, QW], F32)
    T["ccq_out"] = nc.dram_tensor("ccq_out", [1, QW], F32)
    T["cc1_in"] = nc.dram_tensor("cc1_in", [1, KSPLIT], F32)
    T["cc1_out"] = nc.dram_tensor("cc1_out", [1, KSPLIT], F32)
    T["cc2_in"] = nc.dram_tensor("cc2_in", [1, CC2], F32)
    T["cc2_out"] = nc.dram_tensor("cc2_out", [1, CC2], F32)

    with tile.TileContext(nc) as tc:
        _emit(nc, tc, T, has_bias, has_gk)
    nc.compile()
    return nc


def _emit(nc, tc, T, has_bias, has_gk):
    from contextlib import ExitStack
    RG = [[0, 1, 2, 3], [4, 5, 6, 7]]
    es = ExitStack()
    with es:
        keep = es.enter_context(tc.tile_pool(name="keep", bufs=1))
        rows = es.enter_context(tc.tile_pool(name="rows", bufs=1))

        gq_sb = keep.tile([128, 3], F32, tag="gq")
        nc.sync.dma_start(out=gq_sb, in_=T["gq_d"].ap())
        if has_gk:
            gk_sb = keep.tile([128, 3], F32, tag="gk")
            nc.sync.dma_start(out=gk_sb, in_=T["gk_d"].ap())
        ones_sb = keep.tile([128, 1], BF16, tag="ones")
        nc.vector.memset(ones_sb, 1.0)
        eps_sb = keep.tile([128, 1], F32, tag="eps")
        nc.vector.memset(eps_sb, EPS)
        if has_bias:
            b_sb = keep.tile([1, 3, 384], BF16, tag="bqkv")
            nc.sync.dma_start(out=b_sb, in_=T["bqkv"].ap())
            onesrow = keep.tile([1, 512], BF16, tag="onesrow")
            nc.vector.memset(onesrow, 1.0)

        kT = [keep.tile([128, L], BF16, tag=f"kT{h}", name=f"kT{h}")
              for h in range(3)]
        qraw = [keep.tile([128, QW], BF16, tag=f"qraw{h}", name=f"qraw{h}")
                for h in range(3)]
        qT = [keep.tile([128, QW], BF16, tag=f"qT{h}", name=f"qT{h}")
              for h in range(3)]
        ntok_tiles = (L + 127) // 128  # 37
        v_sb = [keep.tile([min(128, L - i * 128), 384], BF16, tag=f"v{i}",
                          name=f"v{i}")
                for i in range(ntok_tiles)]


        # =============== P1: projections + rope + ss partials ===============
        with tc.tile_pool(name="pw", bufs=1) as pw, \
             tc.tile_pool(name="pxk", bufs=2) as pxk, \
             tc.tile_pool(name="ptab", bufs=2) as ptab, \
             tc.tile_pool(name="tmp", bufs=1) as tmp, \
             tc.tile_pool(name="tmp2", bufs=2) as tmp2, \
             tc.tile_pool(name="ppk", bufs=2, space="PSUM") as ppk, \
             tc.tile_pool(name="ppv", bufs=2, space="PSUM") as ppv, \
             tc.tile_pool(name="pps", bufs=2, space="PSUM") as pps:
            xq0 = pxk.tile([128, KC, QTG], BF16, tag="xk")
            nc.sync.dma_start(
                out=xq0,
                in_=T["xTq"].ap()[:, :, 0:QTG].rearrange("c p n -> p c n"))
            cq0 = ptab.tile([64, QTG], BF16, tag="cs")
            nc.sync.dma_start(out=cq0, in_=T["cosq"].ap()[:, 0:QTG])
            sq0 = ptab.tile([64, QTG], BF16, tag="sn")
            nc.sync.dma_start(out=sq0, in_=T["sinq"].ap()[:, 0:QTG])
            wq_c = []
            for kc in range(KC):
                t = pw.tile([128, 384], BF16, tag=f"wq{kc}", name=f"wq{kc}")
                nc.sync.dma_start(out=t, in_=T["wqT"].ap()[kc])
                wq_c.append(t)
            wk_sb = pw.tile([128, KC, 384], BF16, tag="wk")
            nc.sync.dma_start(out=wk_sb, in_=T["wkT"].ap().rearrange("c p n -> p c n"))
            wv_sb = pw.tile([128, KC, 384], BF16, tag="wv")
            nc.sync.dma_start(out=wv_sb, in_=T["wvT"].ap().rearrange("c p n -> p c n"))
            wk_c = [wk_sb[:, kc, :] for kc in range(KC)]
            wv_c = [wv_sb[:, kc, :] for kc in range(KC)]

            def rope_evac(psum, cos_sl, sin_sl, dst, col0, n, g_sl):
                t1 = tmp.tile([64, QTG], F32, tag="t1")
                t2 = tmp.tile([64, QTG], F32, tag="t2")
                t3 = tmp.tile([64, QTG], F32, tag="t3")
                t4 = tmp.tile([64, QTG], F32, tag="t4")
                nc.vector.tensor_mul(t1[:, :n], psum[0:64, :n], cos_sl)
                nc.vector.tensor_mul(t2[:, :n], psum[64:128, :n], sin_sl)
                nc.vector.tensor_mul(t3[:, :n], psum[0:64, :n], sin_sl)
                nc.vector.tensor_mul(t4[:, :n], psum[64:128, :n], cos_sl)
                nc.vector.tensor_sub(dst[0:64, col0:col0 + n], t1[:, :n], t2[:, :n])
                nc.vector.tensor_add(dst[64:128, col0:col0 + n], t3[:, :n], t4[:, :n])
                # ss partial from the roped (pre-gain) values
                sq = tmp2.tile([128, QTG], BF16, tag="sq", bufs=8)
                nc.scalar.square(sq[:, :n], dst[:, col0:col0 + n])
                if g_sl is not None:
                    gtmp = tmp2.tile([128, QTG], BF16, tag="gtmp")
                    nc.vector.tensor_scalar_mul(gtmp[:, :n],
                                                dst[:, col0:col0 + n], g_sl)
                    nc.scalar.copy(out=dst[:, col0:col0 + n], in_=gtmp[:, :n])
                return sq

            # Deferred ss emission: the ones-matmul waits on Square(ACT); by
            # lagging one token-group the PE queue never stalls on it.
            ss_pending = []

            def flush_ss():
                for sqs, ss_ps, n, dram, off in ss_pending:
                    for h, sq in enumerate(sqs):
                        nc.tensor.matmul(ss_ps[:, :n], ones_sb, sq[:, :n],
                                         start=(h == 0), stop=(h == 2))
                    st = tmp2.tile([1, QTG], F32, tag="ssst")
                    nc.vector.tensor_copy(st[:, :n], ss_ps[:, :n])
                    nc.sync.dma_start(out=dram.ap()[:, off:off + n],
                                      in_=st[:, :n])
                ss_pending.clear()

            def kv_group(tg):
                c0 = tg * TG
                n = min(TG, L - c0)
                xk = pxk.tile([128, KC, QTG], BF16, tag="xk")
                nc.sync.dma_start(
                    out=xk[:, :, :n],
                    in_=T["xT"].ap()[:, :, c0:c0 + n].rearrange("c p n -> p c n"))
                ck = ptab.tile([64, QTG], BF16, tag="cs")
                nc.sync.dma_start(out=ck[:, :n], in_=T["cosk"].ap()[:, c0:c0 + n])
                sk = ptab.tile([64, QTG], BF16, tag="sn")
                nc.sync.dma_start(out=sk[:, :n], in_=T["sink"].ap()[:, c0:c0 + n])
                ss_ps = pps.tile([1, QTG], F32, tag="pss")
                sqs = []
                for h in range(3):
                    psk = ppk.tile([128, QTG], F32, tag="pk", bufs=3)
                    for kc in range(KC):
                        nc.tensor.matmul(psk[:, :n],
                                         wk_c[kc][:, h * 128:(h + 1) * 128],
                                         xk[:, kc, :n], start=(kc == 0),
                                         stop=(not has_bias and kc == KC - 1))
                    if has_bias:
                        nc.tensor.matmul(psk[:, :n],
                                         b_sb[:, 1, h * 128:(h + 1) * 128],
                                         onesrow[:, :n], start=False, stop=True)
                    g_sl = gk_sb[:, h:h + 1] if has_gk else None
                    sqs.append(rope_evac(psk, ck[:, :n], sk[:, :n], kT[h], c0, n,
                                         g_sl))
                # V proj from the same x tiles (token-major)
                for j in range(3):
                    vi = tg * 3 + j
                    if vi >= ntok_tiles or vi * 128 >= c0 + n:
                        break
                    rsz = v_sb[vi].shape[0]
                    j0 = vi * 128 - c0
                    psv = ppv.tile([128, 384], F32, tag="pv")
                    for kc in range(KC):
                        nc.tensor.matmul(psv[:rsz, :], xk[:, kc, j0:j0 + rsz],
                                         wv_c[kc], start=(kc == 0),
                                         stop=(not has_bias and kc == KC - 1))
                    if has_bias:
                        nc.tensor.matmul(psv[:rsz, :], onesrow[:, :rsz],
                                         b_sb[:, 2, :], start=False, stop=True)
                    nc.vector.tensor_copy(v_sb[vi], psv[:rsz, :])
                flush_ss()
                if c0 < KSPLIT:
                    ss_pending.append((sqs, ss_ps, n, T["cc1_in"], c0))
                else:
                    ss_pending.append((sqs, ss_ps, n, T["cc2_in"], c0 - KSPLIT))

            ng = (L + TG - 1) // TG  # 13
            ng1 = KSPLIT // TG       # 9 groups before AR-k1

            # --- Q projection first (core's 2340 tokens): 5 groups of 468
            for tg in range(QW // QTG):
                c0 = tg * QTG
                n = QTG
                if tg == 0:
                    xq, cq, sq_t = xq0, cq0, sq0
                else:
                    xq = pxk.tile([128, KC, QTG], BF16, tag="xk")
                    nc.sync.dma_start(
                        out=xq,
                        in_=T["xTq"].ap()[:, :, c0:c0 + n].rearrange("c p n -> p c n"))
                    cq = ptab.tile([64, QTG], BF16, tag="cs")
                    nc.sync.dma_start(out=cq, in_=T["cosq"].ap()[:, c0:c0 + n])
                    sq_t = ptab.tile([64, QTG], BF16, tag="sn")
                    nc.sync.dma_start(out=sq_t, in_=T["sinq"].ap()[:, c0:c0 + n])
                ss_ps = pps.tile([1, QTG], F32, tag="pss")
                sqs = []
                for h in range(3):
                    psq = ppk.tile([128, QTG], F32, tag="pk", bufs=3)
                    for kc in range(KC):
                        nc.tensor.matmul(psq,
                                         wq_c[kc][:, h * 128:(h + 1) * 128],
                                         xq[:, kc, :], start=(kc == 0),
                                         stop=(not has_bias and kc == KC - 1))
                    if has_bias:
                        nc.tensor.matmul(psq, b_sb[:, 0, h * 128:(h + 1) * 128],
                                         onesrow[:, :n], start=False, stop=True)
                    sqs.append(rope_evac(psq, cq, sq_t, qraw[h], c0, n, None))
                flush_ss()
                ss_pending.append((sqs, ss_ps, n, T["ccq_in"], c0))
            flush_ss()

            # =============== AR-q ===========================================
            nc.gpsimd.collective_compute(
                "AllReduce", mybir.AluOpType.add, replica_groups=RG,
                ins=[T["ccq_in"].ap().opt()], outs=[T["ccq_out"].ap().opt()])

            # rms-q chain (hides under the K/V pass below)
            ssq_all = rows.tile([1, QW], F32, tag="ssqall")
            nc.sync.dma_start(out=ssq_all, in_=T["ccq_out"].ap())
            rq_sq = rows.tile([1, QW], F32, tag="rqsq")
            nc.scalar.activation(rq_sq, ssq_all,
                                 mybir.ActivationFunctionType.Sqrt,
                                 scale=float(1.0 / D), bias=eps_sb[0:1, :])
            rq_row = rows.tile([1, QW], F32, tag="rqrow")
            nc.vector.reciprocal_approx_fast(rq_row, rq_sq)
            nc.vector.tensor_scalar_mul(rq_row, rq_row, ISD)
            rqrep = rows.tile([128, QW], F32, tag="rqrep")
            nc.gpsimd.partition_broadcast(rqrep, rq_row)
            for h in range(3):
                nc.vector.scalar_tensor_tensor(
                    out=qT[h], in0=qraw[h], scalar=gq_sb[:, h:h + 1], in1=rqrep,
                    op0=mybir.AluOpType.mult, op1=mybir.AluOpType.mult)

            # --- K/V head groups 0..8
            for tg in range(ng1):
                kv_group(tg)
            flush_ss()

            # =============== AR-k1: ss_k[0:KSPLIT] ==========================
            nc.gpsimd.collective_compute(
                "AllReduce", mybir.AluOpType.add, replica_groups=RG,
                ins=[T["cc1_in"].ap().opt()], outs=[T["cc1_out"].ap().opt()])

            # --- K/V tail (tokens KSPLIT:L) overlaps AR-k1
            for tg in range(ng1, ng):
                kv_group(tg)
            flush_ss()

            # =============== AR-k2: ss_k[KSPLIT:] ===========================
            zr = tmp2.tile([1, 64], F32, tag="zr")
            nc.vector.memset(zr, 0.0)
            nc.sync.dma_start(out=T["cc2_in"].ap()[:, L - KSPLIT:CC2],
                              in_=zr[:, :CC2 - (L - KSPLIT)])
            nc.gpsimd.collective_compute(
                "AllReduce", mybir.AluOpType.add, replica_groups=RG,
                ins=[T["cc2_in"].ap().opt()], outs=[T["cc2_out"].ap().opt()])

            # k segment 1: partition-major gather [128, KCOL1]
            ccr1 = rows.tile([128, KCOL1], F32, tag="ccr1")
            nc.sync.dma_start(out=ccr1, in_=bass.AP(
                tensor=T["cc1_out"].ap().tensor, offset=0,
                ap=[[1, 128], [128, KCOL1]]))
            ra1 = rows.tile([128, KCOL1], F32, tag="ra1")
            nc.scalar.activation(ra1, ccr1, mybir.ActivationFunctionType.Sqrt,
                                 scale=float(1.0 / D), bias=eps_sb)
            rmsk_a = rows.tile([128, KCOL1], F32, tag="rmska")
            nc.vector.reciprocal(rmsk_a, ra1)

            # k segment 2 (from AR#2): [128, KCOL2]
            ccr2 = rows.tile([128, KCOL2], F32, tag="ccr2")
            nc.sync.dma_start(out=ccr2, in_=bass.AP(
                tensor=T["cc2_out"].ap().tensor, offset=0,
                ap=[[1, 128], [128, KCOL2]]))
            ra2 = rows.tile([128, KCOL2], F32, tag="ra2")
            nc.scalar.activation(ra2, ccr2, mybir.ActivationFunctionType.Sqrt,
                                 scale=float(1.0 / D), bias=eps_sb)
            rmsk_b = rows.tile([128, KCOL2], F32, tag="rmskb")
            nc.vector.reciprocal(rmsk_b, ra2)

        # =============== P3: attention + O projection ===============
        # The ones/PV matmuls wait on the exp (ACT); emit them a few k-tiles
        # late so the in-order PE queue never stalls on the ACT latency.
        with tc.tile_pool(name="pat", bufs=2) as pat, \
             tc.tile_pool(name="pps2", bufs=2, space="PSUM") as pps2, \
             tc.tile_pool(name="ppo", bufs=1, space="PSUM") as ppo, \
             tc.tile_pool(name="ppm", bufs=1, space="PSUM") as ppm:
            LOOK = 3
            wo_sb = keep.tile([128, 3, KC, 128], BF16, tag="wo")
            nc.sync.dma_start(out=wo_sb,
                              in_=T["woT"].ap().rearrange("h c p n -> p h c n"))

            def rmsk_sl(kt, kk):
                if kt < KCOL1:
                    return rmsk_a[:kk, kt:kt + 1]
                return rmsk_b[:kk, kt - KCOL1:kt - KCOL1 + 1]

            for qf in range(NFR):
                krange = FRT * (qf + 1)
                nkt = (krange + 127) // 128
                attn = []
                for h in range(3):
                    ps_o = ppo.tile([128, 2, 512], F32, tag="po")
                    ps_d = ppm.tile([1, 2, 512], F32, tag="pmisc")
                    pending = []      # (kt, kk, p_sb) awaiting PV
                    dpend = []        # full exp tiles awaiting quad-add
                    dgroups = []      # (tile, kk) for the denominator matmuls

                    def flush_one():
                        kt0, kk0, p0 = pending.pop(0)
                        for b in range(2):
                            nc.tensor.matmul(ps_o[:, b, 0:390],
                                             v_sb[kt0][:kk0, h * 128:(h + 1) * 128],
                                             p0[:kk0, b, :],
                                             start=(kt0 == 0),
                                             stop=(kt0 == nkt - 1))

                    def pop_dpend():
                        if len(dpend) >= 2:
                            a1 = pat.tile([128, 2, 390], BF16, tag="padd",
                                          bufs=6)
                            nc.vector.tensor_add(a1, dpend[0], dpend[1])
                            if len(dpend) == 4:
                                a2 = pat.tile([128, 2, 390], BF16, tag="padd", bufs=6)
                                nc.vector.tensor_add(a2, dpend[2], dpend[3])
                                a3 = pat.tile([128, 2, 390], BF16, tag="padd", bufs=6)
                                nc.vector.tensor_add(a3, a1, a2)
                                dgroups.append((a3, 128))
                            elif len(dpend) == 3:
                                a3 = pat.tile([128, 2, 390], BF16, tag="padd", bufs=6)
                                nc.vector.tensor_add(a3, a1, dpend[2])
                                dgroups.append((a3, 128))
                            else:
                                dgroups.append((a1, 128))
                        elif dpend:
                            dgroups.append((dpend[0], 128))
                        dpend.clear()

                    for kt in range(nkt):
                        kk = min(128, krange - kt * 128)
                        ps_s = pps2.tile([128, 2, 512], F32, tag="ps")
                        for b in range(2):
                            nc.tensor.matmul(
                                ps_s[:kk, b, 0:390],
                                kT[h][:, kt * 128:kt * 128 + kk],
                                qT[h][:, qf * QH + b * 390: qf * QH + (b + 1) * 390],
                                start=True, stop=True)
                        p_sb = pat.tile([128, 2, 390], BF16, tag="p", bufs=10)
                        nc.scalar.activation(p_sb[:kk], ps_s[:kk, :, 0:390],
                                             mybir.ActivationFunctionType.Exp,
                                             scale=rmsk_sl(kt, kk))
                        pending.append((kt, kk, p_sb))
                        if kk == 128:
                            dpend.append(p_sb)
                            if len(dpend) == 4:
                                pop_dpend()
                        else:
                            pop_dpend()
                            dgroups.append((p_sb, kk))
                        if len(pending) > LOOK:
                            flush_one()
                    while pending:
                        flush_one()
                    pop_dpend()
                    # denominator matmuls, all deps long satisfied by now
                    for gi, (gt, gk) in enumerate(dgroups):
                        for b in range(2):
                            nc.tensor.matmul(ps_d[:, b, 0:390], ones_sb[:gk, :],
                                             gt[:gk, b, :],
                                             start=(gi == 0),
                                             stop=(gi == len(dgroups) - 1))
                    drecip = pat.tile([1, 2, 390], F32, tag="dr")
                    nc.vector.reciprocal_approx_fast(drecip, ps_d[:, :, 0:390])
                    drep = pat.tile([128, 2, 390], F32, tag="drep")
                    nc.gpsimd.partition_broadcast(
                        drep.rearrange("p a b -> p (a b)"),
                        drecip.rearrange("p a b -> p (a b)"))
                    at = pat.tile([128, 2, 390], BF16, tag=f"at{h}")
                    nc.vector.tensor_mul(at, ps_o[:, :, 0:390], drep)
                    attn.append(at)
                for oc in range(KC):
                    ps_oc = ppm.tile([128, 2, 512], F32, tag="pmisc")
                    for h in range(3):
                        for b in range(2):
                            nc.tensor.matmul(ps_oc[:, b, 0:390], wo_sb[:, h, oc, :],
                                             attn[h][:, b, :],
                                             start=(h == 0), stop=(h == 2))
                    o_sb = pat.tile([128, 2, 390], F16, tag="osb")
                    nc.vector.tensor_copy(o_sb, ps_oc[:, :, 0:390])
                    nc.sync.dma_start(
                        out=T["outOT"].ap()[oc * 128:(oc + 1) * 128,
                                            qf * QH:(qf + 1) * QH],
                        in_=o_sb.rearrange("p a b -> p (a b)"))


def _prep(inputs):
    g = {k: np.asarray(v) for k, v in inputs.items()}
    x = g["x"].astype(np.float32)
    assert x.shape == (1, L, D), x.shape
    has_bias = any(np.any(g[k].astype(np.float64)) for k in ("bq", "bk", "bv"))
    has_gk = not np.all(g["gk"] == 1.0)

    perm = np.empty(D, np.int64)
    for h in range(NH):
        base = h * HD
        perm[base:base + 64] = base + 2 * np.arange(64)
        perm[base + 64:base + 128] = base + 2 * np.arange(64) + 1

    cos_t, sin_t = _rope_tables(g["freqs"].astype(np.float32))  # [64, L]

    xT = np.ascontiguousarray(x[0].T).astype(BF)                # [D, L]
    xT_dev = np.ascontiguousarray(xT.reshape(KC, 128, L))

    wq_p = g["wq"].astype(np.float32)[perm]   # q feature j = row j of wq
    wk_p = g["wk"].astype(np.float32)[perm]
    gq_p = g["gq"].astype(np.float32)[perm]
    gk_p = g["gk"].astype(np.float32)[perm]

    cos_bf = cos_t.astype(BF)
    sin_bf = sin_t.astype(BF)

    in_maps = []
    qcols_all = []
    for c in range(8):
        gidx, s = c % 4, c // 4
        heads = [3 * gidx, 3 * gidx + 1, 3 * gidx + 2]
        cols = np.concatenate([np.arange(h * HD, (h + 1) * HD) for h in heads])
        qcols = np.concatenate(
            [np.arange(f * FRT + s * QH, f * FRT + (s + 1) * QH)
             for f in range(NFR)])
        qcols_all.append(qcols)

        woT = np.empty((3, KC, 128, 128), np.float32)
        for hl, h in enumerate(heads):
            for oc in range(KC):
                woT[hl, oc] = g["wo"].astype(np.float32)[
                    oc * 128:(oc + 1) * 128, h * HD:(h + 1) * HD].T
        m = dict(
            xT=xT_dev,
            xTq=np.ascontiguousarray(xT_dev[:, :, qcols]),
            wqT=np.ascontiguousarray(wq_p[cols].T).astype(BF).reshape(KC, 128, 384),
            wkT=np.ascontiguousarray(wk_p[cols].T).astype(BF).reshape(KC, 128, 384),
            wvT=np.ascontiguousarray(
                g["wv"].astype(np.float32)[cols].T).astype(BF).reshape(KC, 128, 384),
            woT=woT.astype(BF),
            cosk=cos_bf, sink=sin_bf,
            cosq=np.ascontiguousarray(cos_bf[:, qcols]),
            sinq=np.ascontiguousarray(sin_bf[:, qcols]),
            gq_d=np.ascontiguousarray(gq_p[cols].reshape(3, 128).T),
        )
        if has_gk:
            m["gk_d"] = np.ascontiguousarray(gk_p[cols].reshape(3, 128).T)
        if has_bias:
            bq_p = g["bq"].astype(np.float32)[perm]
            bk_p = g["bk"].astype(np.float32)[perm]
            m["bqkv"] = np.stack(
                [bq_p[cols], bk_p[cols], g["bv"].astype(np.float32)[cols]]
            ).reshape(1, 3, 384).astype(BF)
        m = {k: np.ascontiguousarray(v) for k, v in m.items()}
        in_maps.append(m)
    return in_maps, qcols_all, (has_bias, has_gk), g


def kernel(**inputs):
    in_maps, qcols_all, flags, g = _prep(inputs)
    if flags not in _nc_cache:
        _nc_cache[flags] = _build(*flags)
    nc = _nc_cache[flags]
    res = bass_utils.run_bass_kernel_spmd(nc, in_maps, core_ids=list(range(8)))
    out = np.zeros((D, L), np.float32)
    for c in range(8):
        out[:, qcols_all[c]] += res.results[c]["outOT"].astype(np.float32)
    out = out.T[None, :, :] + g["bo"].astype(np.float32)
    return out.astype(np.float32)


# revision 13
# speedup vs baseline: 1.0438x; 1.0080x over previous
"""CausalWanSelfAttention on 8 trn2 NeuronCores (Bass/Tile, SPMD).

Sharding: 4 head-groups (3 heads each) x 2 query-halves (780 q-rows per frame).
The frame mask (F=3, sink=1, local=2) is exactly frame-block-causal, so
attention is dense per (qframe, kframe<=qframe) block.

v2 pipeline per device:
  One pass over x computes K (feature-major, rope fused into PSUM evac) AND
  V (token-major) from shared x tiles; Q proj (core's 2340 tokens) likewise
  rope-fused.  Sum-of-squares partials accumulate in SBUF row tiles and go
  out as TWO mask-free AllReduces over the same-query-half 4-core groups
  ({0,2,4,6}/{1,3,5,7}): AR#1 = [ss_q | ss_k(0:3456)] issued before the K/V
  tail (tokens 3456:4680) so the tail + frame-0/1 attention hide AR latency;
  AR#2 = ss_k(3456:) hidden under frame-0/1 attention.  rms_k folds into the
  exp scale (per-partition in transposed-score layout), rms_q (and 1/sqrt(d))
  folds into qT.  Transposed-score flash attention without max-subtraction;
  softmax denominators via ones-column matmuls over quad-added exp tiles;
  per-core partial O-projection (fp16) summed on the host.
"""
import sys
sys.path.insert(0, '/opt/trn_rl_repo')

import numpy as np
import ml_dtypes

import concourse.bass as bass
import concourse.mybir as mybir
import concourse.tile as tile
from concourse import bacc, bass_utils

F32 = mybir.dt.float32
F16 = mybir.dt.float16
BF16 = mybir.dt.bfloat16
BF = ml_dtypes.bfloat16

L, D, NH, HD = 4680, 1536, 12, 128
NFR, FRT = 3, 1560          # frames, tokens per frame
QH = 780                    # q rows per (core, frame)
QW = 3 * QH                 # 2340 q tokens per core
KC = 12                     # contraction chunks of 128
TG = 384                    # token group for k/v proj (12 full + 72 runt)
QTG = 468                   # token group for q proj (5 exact)
KSPLIT = 3456               # k tokens covered by AR#1 (groups 0..8)
KCOL1 = KSPLIT // 128       # 27
LPAD = 4736                 # 37*128
KCOL2 = (LPAD - KSPLIT) // 128  # 10 (last col has 72 valid)
CC1 = QW + KSPLIT           # 5796 f32
CC2 = LPAD - KSPLIT         # 1280 f32 (tail zero-padded)
EPS = 1e-6
ISD = float(1.0 / np.sqrt(HD))

_nc_cache = {}


def _rope_tables(freqs):
    """cos/sin [64, L] float32 per rope pair, matching reference _rope_table."""
    c = freqs.shape[1]           # 64
    s0 = c - 2 * (c // 3)        # 22
    s1 = c // 3                  # 21
    Fr, H, W = NFR, 30, 52
    fr = np.zeros((Fr, H, W, c, 2), np.float32)
    fr[:, :, :, :s0] = freqs[:Fr, :s0].reshape(Fr, 1, 1, s0, 2)
    fr[:, :, :, s0:s0 + s1] = freqs[:H, s0:s0 + s1].reshape(1, H, 1, s1, 2)
    fr[:, :, :, s0 + s1:] = freqs[:W, s0 + s1:].reshape(1, 1, W, s1, 2)
    fr = fr.reshape(L, c, 2)
    return fr[:, :, 0].T.copy(), fr[:, :, 1].T.copy()  # [64, L] each


def _build(has_bias, has_gk):
    nc = bacc.Bacc(trn_type="TRN2", debug=False, num_devices=8)
    ExtIn = dict(kind="ExternalInput")
    T = {}
    T["xT"] = nc.dram_tensor("xT", [KC, 128, L], BF16, **ExtIn)
    T["xTq"] = nc.dram_tensor("xTq", [KC, 128, QW], BF16, **ExtIn)
    T["wqT"] = nc.dram_tensor("wqT", [KC, 128, 384], BF16, **ExtIn)
    T["wkT"] = nc.dram_tensor("wkT", [KC, 128, 384], BF16, **ExtIn)
    T["wvT"] = nc.dram_tensor("wvT", [KC, 128, 384], BF16, **ExtIn)
    T["woT"] = nc.dram_tensor("woT", [3, KC, 128, 128], BF16, **ExtIn)
    T["cosk"] = nc.dram_tensor("cosk", [64, L], BF16, **ExtIn)
    T["sink"] = nc.dram_tensor("sink", [64, L], BF16, **ExtIn)
    T["cosq"] = nc.dram_tensor("cosq", [64, QW], BF16, **ExtIn)
    T["sinq"] = nc.dram_tensor("sinq", [64, QW], BF16, **ExtIn)
    T["gq_d"] = nc.dram_tensor("gq_d", [128, 3], F32, **ExtIn)
    if has_gk:
        T["gk_d"] = nc.dram_tensor("gk_d", [128, 3], F32, **ExtIn)
    if has_bias:
        T["bqkv"] = nc.dram_tensor("bqkv", [1, 3, 384], BF16, **ExtIn)
    T["outOT"] = nc.dram_tensor("outOT", [D, QW], F16, kind="ExternalOutput")
    T["ccq_in"] = nc.dram_tensor("ccq_in", [1, QW], F32)
    T["ccq_out"] = nc.dram_tensor("ccq_out", [1, QW], F32)
    T["cc1_in"] = nc.dram_tensor("cc1_in", [1, KSPLIT], F32)
    T["cc1_out"] = nc.dram_tensor("cc1_out", [1, KSPLIT], F32)
    T["cc2_in"] = nc.dram_tensor("cc2_in", [1, CC2], F32)
    T["cc2_out"] = nc.dram_tensor("cc2_out", [1, CC2], F32)

    with tile.TileContext(nc) as tc:
        _emit(nc, tc, T, has_bias, has_gk)
    nc.compile()
    return nc


def _emit(nc, tc, T, has_bias, has_gk):
    from contextlib import ExitStack
    RG = [[0, 1, 2, 3], [4, 5, 6, 7]]
    es = ExitStack()
    with es:
        keep = es.enter_context(tc.tile_pool(name="keep", bufs=1))
        rows = es.enter_context(tc.tile_pool(name="rows", bufs=1))

        gq_sb = keep.tile([128, 3], F32, tag="gq")
        nc.sync.dma_start(out=gq_sb, in_=T["gq_d"].ap())
        if has_gk:
            gk_sb = keep.tile([128, 3], F32, tag="gk")
            nc.sync.dma_start(out=gk_sb, in_=T["gk_d"].ap())
        ones_sb = keep.tile([128, 1], BF16, tag="ones")
        nc.vector.memset(ones_sb, 1.0)
        eps_sb = keep.tile([128, 1], F32, tag="eps")
        nc.vector.memset(eps_sb, EPS)
        if has_bias:
            b_sb = keep.tile([1, 3, 384], BF16, tag="bqkv")
            nc.sync.dma_start(out=b_sb, in_=T["bqkv"].ap())
            onesrow = keep.tile([1, 512], BF16, tag="onesrow")
            nc.vector.memset(onesrow, 1.0)

        kT = [keep.tile([128, L], BF16, tag=f"kT{h}", name=f"kT{h}")
              for h in range(3)]
        qraw = [keep.tile([128, QW], BF16, tag=f"qraw{h}", name=f"qraw{h}")
                for h in range(3)]
        qT = [keep.tile([128, QW], BF16, tag=f"qT{h}", name=f"qT{h}")
              for h in range(3)]
        ntok_tiles = (L + 127) // 128  # 37
        v_sb = [keep.tile([min(128, L - i * 128), 384], BF16, tag=f"v{i}",
                          name=f"v{i}")
                for i in range(ntok_tiles)]


        # =============== P1: projections + rope + ss partials ===============
        with tc.tile_pool(name="pw", bufs=1) as pw, \
             tc.tile_pool(name="pxk", bufs=2) as pxk, \
             tc.tile_pool(name="ptab", bufs=2) as ptab, \
             tc.tile_pool(name="tmp", bufs=1) as tmp, \
             tc.tile_pool(name="tmp2", bufs=2) as tmp2, \
             tc.tile_pool(name="ppk", bufs=2, space="PSUM") as ppk, \
             tc.tile_pool(name="ppv", bufs=2, space="PSUM") as ppv, \
             tc.tile_pool(name="pps", bufs=2, space="PSUM") as pps:
            xq0 = pxk.tile([128, KC, QTG], BF16, tag="xk")
            nc.sync.dma_start(
                out=xq0,
                in_=T["xTq"].ap()[:, :, 0:QTG].rearrange("c p n -> p c n"))
            cq0 = ptab.tile([64, QTG], BF16, tag="cs")
            nc.sync.dma_start(out=cq0, in_=T["cosq"].ap()[:, 0:QTG])
            sq0 = ptab.tile([64, QTG], BF16, tag="sn")
            nc.sync.dma_start(out=sq0, in_=T["sinq"].ap()[:, 0:QTG])
            wq_c = []
            for kc in range(KC):
                t = pw.tile([128, 384], BF16, tag=f"wq{kc}", name=f"wq{kc}")
                nc.sync.dma_start(out=t, in_=T["wqT"].ap()[kc])
                wq_c.append(t)
            wk_sb = pw.tile([128, KC, 384], BF16, tag="wk")
            nc.sync.dma_start(out=wk_sb, in_=T["wkT"].ap().rearrange("c p n -> p c n"))
            wv_sb = pw.tile([128, KC, 384], BF16, tag="wv")
            nc.sync.dma_start(out=wv_sb, in_=T["wvT"].ap().rearrange("c p n -> p c n"))
            wk_c = [wk_sb[:, kc, :] for kc in range(KC)]
            wv_c = [wv_sb[:, kc, :] for kc in range(KC)]

            def rope_evac(psum, cos_sl, sin_sl, dst, col0, n, g_sl):
                t1 = tmp.tile([64, QTG], F32, tag="t1")
                t2 = tmp.tile([64, QTG], F32, tag="t2")
                t3 = tmp.tile([64, QTG], F32, tag="t3")
                t4 = tmp.tile([64, QTG], F32, tag="t4")
                nc.vector.tensor_mul(t1[:, :n], psum[0:64, :n], cos_sl)
                nc.vector.tensor_mul(t2[:, :n], psum[64:128, :n], sin_sl)
                nc.vector.tensor_mul(t3[:, :n], psum[0:64, :n], sin_sl)
                nc.vector.tensor_mul(t4[:, :n], psum[64:128, :n], cos_sl)
                nc.vector.tensor_sub(dst[0:64, col0:col0 + n], t1[:, :n], t2[:, :n])
                nc.vector.tensor_add(dst[64:128, col0:col0 + n], t3[:, :n], t4[:, :n])
                # ss partial from the roped (pre-gain) values
                sq = tmp2.tile([128, QTG], BF16, tag="sq", bufs=8)
                nc.scalar.square(sq[:, :n], dst[:, col0:col0 + n])
                if g_sl is not None:
                    gtmp = tmp2.tile([128, QTG], BF16, tag="gtmp")
                    nc.vector.tensor_scalar_mul(gtmp[:, :n],
                                                dst[:, col0:col0 + n], g_sl)
                    nc.scalar.copy(out=dst[:, col0:col0 + n], in_=gtmp[:, :n])
                return sq

            # Deferred ss emission: the ones-matmul waits on Square(ACT); by
            # lagging one token-group the PE queue never stalls on it.
            ss_pending = []

            def flush_ss():
                for sqs, ss_ps, n, dram, off in ss_pending:
                    for h, sq in enumerate(sqs):
                        nc.tensor.matmul(ss_ps[:, :n], ones_sb, sq[:, :n],
                                         start=(h == 0), stop=(h == 2))
                    st = tmp2.tile([1, QTG], F32, tag="ssst")
                    nc.vector.tensor_copy(st[:, :n], ss_ps[:, :n])
                    nc.sync.dma_start(out=dram.ap()[:, off:off + n],
                                      in_=st[:, :n])
                ss_pending.clear()

            def kv_group(tg):
                c0 = tg * TG
                n = min(TG, L - c0)
                xk = pxk.tile([128, KC, QTG], BF16, tag="xk")
                nc.sync.dma_start(
                    out=xk[:, :, :n],
                    in_=T["xT"].ap()[:, :, c0:c0 + n].rearrange("c p n -> p c n"))
                ck = ptab.tile([64, QTG], BF16, tag="cs")
                nc.sync.dma_start(out=ck[:, :n], in_=T["cosk"].ap()[:, c0:c0 + n])
                sk = ptab.tile([64, QTG], BF16, tag="sn")
                nc.sync.dma_start(out=sk[:, :n], in_=T["sink"].ap()[:, c0:c0 + n])
                ss_ps = pps.tile([1, QTG], F32, tag="pss")
                sqs = []
                for h in range(3):
                    psk = ppk.tile([128, QTG], F32, tag="pk", bufs=3)
                    for kc in range(KC):
                        nc.tensor.matmul(psk[:, :n],
                                         wk_c[kc][:, h * 128:(h + 1) * 128],
                                         xk[:, kc, :n], start=(kc == 0),
                                         stop=(not has_bias and kc == KC - 1))
                    if has_bias:
                        nc.tensor.matmul(psk[:, :n],
                                         b_sb[:, 1, h * 128:(h + 1) * 128],
                                         onesrow[:, :n], start=False, stop=True)
                    g_sl = gk_sb[:, h:h + 1] if has_gk else None
                    sqs.append(rope_evac(psk, ck[:, :n], sk[:, :n], kT[h], c0, n,
                                         g_sl))
                # V proj from the same x tiles (token-major)
                for j in range(3):
                    vi = tg * 3 + j
                    if vi >= ntok_tiles or vi * 128 >= c0 + n:
                        break
                    rsz = v_sb[vi].shape[0]
                    j0 = vi * 128 - c0
                    psv = ppv.tile([128, 384], F32, tag="pv")
                    for kc in range(KC):
                        nc.tensor.matmul(psv[:rsz, :], xk[:, kc, j0:j0 + rsz],
                                         wv_c[kc], start=(kc == 0),
                                         stop=(not has_bias and kc == KC - 1))
                    if has_bias:
                        nc.tensor.matmul(psv[:rsz, :], onesrow[:, :rsz],
                                         b_sb[:, 2, :], start=False, stop=True)
                    nc.vector.tensor_copy(v_sb[vi], psv[:rsz, :])
                flush_ss()
                if c0 < KSPLIT:
                    ss_pending.append((sqs, ss_ps, n, T["cc1_in"], c0))
                else:
                    ss_pending.append((sqs, ss_ps, n, T["cc2_in"], c0 - KSPLIT))

            ng = (L + TG - 1) // TG  # 13
            ng1 = KSPLIT // TG       # 9 groups before AR-k1

            # --- Q projection first (core's 2340 tokens): 5 groups of 468
            for tg in range(QW // QTG):
                c0 = tg * QTG
                n = QTG
                if tg == 0:
                    xq, cq, sq_t = xq0, cq0, sq0
                else:
                    xq = pxk.tile([128, KC, QTG], BF16, tag="xk")
                    nc.sync.dma_start(
                        out=xq,
                        in_=T["xTq"].ap()[:, :, c0:c0 + n].rearrange("c p n -> p c n"))
                    cq = ptab.tile([64, QTG], BF16, tag="cs")
                    nc.sync.dma_start(out=cq, in_=T["cosq"].ap()[:, c0:c0 + n])
                    sq_t = ptab.tile([64, QTG], BF16, tag="sn")
                    nc.sync.dma_start(out=sq_t, in_=T["sinq"].ap()[:, c0:c0 + n])
                ss_ps = pps.tile([1, QTG], F32, tag="pss")
                sqs = []
                for h in range(3):
                    psq = ppk.tile([128, QTG], F32, tag="pk", bufs=3)
                    for kc in range(KC):
                        nc.tensor.matmul(psq,
                                         wq_c[kc][:, h * 128:(h + 1) * 128],
                                         xq[:, kc, :], start=(kc == 0),
                                         stop=(not has_bias and kc == KC - 1))
                    if has_bias:
                        nc.tensor.matmul(psq, b_sb[:, 0, h * 128:(h + 1) * 128],
                                         onesrow[:, :n], start=False, stop=True)
                    sqs.append(rope_evac(psq, cq, sq_t, qraw[h], c0, n, None))
                flush_ss()
                ss_pending.append((sqs, ss_ps, n, T["ccq_in"], c0))
            flush_ss()

            # =============== AR-q ===========================================
            nc.gpsimd.collective_compute(
                "AllReduce", mybir.AluOpType.add, replica_groups=RG,
                ins=[T["ccq_in"].ap().opt()], outs=[T["ccq_out"].ap().opt()])

            # rms-q chain (hides under the K/V pass below)
            ssq_all = rows.tile([1, QW], F32, tag="ssqall")
            nc.sync.dma_start(out=ssq_all, in_=T["ccq_out"].ap())
            rq_sq = rows.tile([1, QW], F32, tag="rqsq")
            nc.scalar.activation(rq_sq, ssq_all,
                                 mybir.ActivationFunctionType.Sqrt,
                                 scale=float(1.0 / D), bias=eps_sb[0:1, :])
            rq_row = rows.tile([1, QW], F32, tag="rqrow")
            nc.vector.reciprocal_approx_fast(rq_row, rq_sq)
            nc.vector.tensor_scalar_mul(rq_row, rq_row, ISD)
            rqrep = rows.tile([128, QW], F32, tag="rqrep")
            nc.gpsimd.partition_broadcast(rqrep, rq_row)
            for h in range(3):
                nc.vector.scalar_tensor_tensor(
                    out=qT[h], in0=qraw[h], scalar=gq_sb[:, h:h + 1], in1=rqrep,
                    op0=mybir.AluOpType.mult, op1=mybir.AluOpType.mult)

            # --- K/V head groups 0..8
            for tg in range(ng1):
                kv_group(tg)
            flush_ss()

            # =============== AR-k1: ss_k[0:KSPLIT] ==========================
            nc.gpsimd.collective_compute(
                "AllReduce", mybir.AluOpType.add, replica_groups=RG,
                ins=[T["cc1_in"].ap().opt()], outs=[T["cc1_out"].ap().opt()])

            # --- K/V tail (tokens KSPLIT:L) overlaps AR-k1
            for tg in range(ng1, ng):
                kv_group(tg)
            flush_ss()

            # =============== AR-k2: ss_k[KSPLIT:] ===========================
            zr = tmp2.tile([1, 64], F32, tag="zr")
            nc.vector.memset(zr, 0.0)
            nc.sync.dma_start(out=T["cc2_in"].ap()[:, L - KSPLIT:CC2],
                              in_=zr[:, :CC2 - (L - KSPLIT)])
            nc.gpsimd.collective_compute(
                "AllReduce", mybir.AluOpType.add, replica_groups=RG,
                ins=[T["cc2_in"].ap().opt()], outs=[T["cc2_out"].ap().opt()])

            # k segment 1: partition-major gather [128, KCOL1]
            ccr1 = rows.tile([128, KCOL1], F32, tag="ccr1")
            nc.sync.dma_start(out=ccr1, in_=bass.AP(
                tensor=T["cc1_out"].ap().tensor, offset=0,
                ap=[[1, 128], [128, KCOL1]]))
            ra1 = rows.tile([128, KCOL1], F32, tag="ra1")
            nc.scalar.activation(ra1, ccr1, mybir.ActivationFunctionType.Sqrt,
                                 scale=float(1.0 / D), bias=eps_sb)
            rmsk_a = rows.tile([128, KCOL1], F32, tag="rmska")
            nc.vector.reciprocal(rmsk_a, ra1)

            # k segment 2 (from AR#2): [128, KCOL2]
            ccr2 = rows.tile([128, KCOL2], F32, tag="ccr2")
            nc.sync.dma_start(out=ccr2, in_=bass.AP(
                tensor=T["cc2_out"].ap().tensor, offset=0,
                ap=[[1, 128], [128, KCOL2]]))
            ra2 = rows.tile([128, KCOL2], F32, tag="ra2")
            nc.scalar.activation(ra2, ccr2, mybir.ActivationFunctionType.Sqrt,
                                 scale=float(1.0 / D), bias=eps_sb)
            rmsk_b = rows.tile([128, KCOL2], F32, tag="rmskb")
            nc.vector.reciprocal(rmsk_b, ra2)

        # =============== P3: attention + O projection ===============
        # The ones/PV matmuls wait on the exp (ACT); emit them a few k-tiles
        # late so the in-order PE queue never stalls on the ACT latency.
        with tc.tile_pool(name="pat", bufs=2) as pat, \
             tc.tile_pool(name="pps2", bufs=2, space="PSUM") as pps2, \
             tc.tile_pool(name="ppo", bufs=1, space="PSUM") as ppo, \
             tc.tile_pool(name="ppm", bufs=1, space="PSUM") as ppm:
            LOOK = 3
            wo_sb = keep.tile([128, 3, KC, 128], BF16, tag="wo")
            nc.sync.dma_start(out=wo_sb,
                              in_=T["woT"].ap().rearrange("h c p n -> p h c n"))

            def rmsk_sl(kt, kk):
                if kt < KCOL1:
                    return rmsk_a[:kk, kt:kt + 1]
                return rmsk_b[:kk, kt - KCOL1:kt - KCOL1 + 1]

            for qf in range(NFR):
                krange = FRT * (qf + 1)
                nkt = (krange + 127) // 128
                attn = []
                for h in range(3):
                    ps_o = ppo.tile([128, 2, 512], F32, tag="po")
                    ps_d = ppm.tile([1, 2, 512], F32, tag="pmisc")
                    pending = []      # (kt, kk, p_sb) awaiting PV
                    dpend = []        # full exp tiles awaiting quad-add
                    dgroups = []      # (tile, kk) for the denominator matmuls

                    def flush_one():
                        kt0, kk0, p0 = pending.pop(0)
                        for b in range(2):
                            nc.tensor.matmul(ps_o[:, b, 0:390],
                                             v_sb[kt0][:kk0, h * 128:(h + 1) * 128],
                                             p0[:kk0, b, :],
                                             start=(kt0 == 0),
                                             stop=(kt0 == nkt - 1))

                    def pop_dpend():
                        if len(dpend) >= 2:
                            a1 = pat.tile([128, 2, 390], BF16, tag="padd",
                                          bufs=6)
                            nc.vector.tensor_add(a1, dpend[0], dpend[1])
                            if len(dpend) == 4:
                                a2 = pat.tile([128, 2, 390], BF16, tag="padd", bufs=6)
                                nc.vector.tensor_add(a2, dpend[2], dpend[3])
                                a3 = pat.tile([128, 2, 390], BF16, tag="padd", bufs=6)
                                nc.vector.tensor_add(a3, a1, a2)
                                dgroups.append((a3, 128))
                            elif len(dpend) == 3:
                                a3 = pat.tile([128, 2, 390], BF16, tag="padd", bufs=6)
                                nc.vector.tensor_add(a3, a1, dpend[2])
                                dgroups.append((a3, 128))
                            else:
                                dgroups.append((a1, 128))
                        elif dpend:
                            dgroups.append((dpend[0], 128))
                        dpend.clear()

                    for kt in range(nkt):
                        kk = min(128, krange - kt * 128)
                        ps_s = pps2.tile([128, 2, 512], F32, tag="ps")
                        for b in range(2):
                            nc.tensor.matmul(
                                ps_s[:kk, b, 0:390],
                                kT[h][:, kt * 128:kt * 128 + kk],
                                qT[h][:, qf * QH + b * 390: qf * QH + (b + 1) * 390],
                                start=True, stop=True)
                        p_sb = pat.tile([128, 2, 390], BF16, tag="p", bufs=10)
                        nc.scalar.activation(p_sb[:kk], ps_s[:kk, :, 0:390],
                                             mybir.ActivationFunctionType.Exp,
                                             scale=rmsk_sl(kt, kk))
                        pending.append((kt, kk, p_sb))
                        if kk == 128:
                            dpend.append(p_sb)
                            if len(dpend) == 4:
                                pop_dpend()
                        else:
                            pop_dpend()
                            dgroups.append((p_sb, kk))
                        if len(pending) > LOOK:
                            flush_one()
                    while pending:
                        flush_one()
                    pop_dpend()
                    # denominator matmuls, all deps long satisfied by now
                    for gi, (gt, gk) in enumerate(dgroups):
                        for b in range(2):
                            nc.tensor.matmul(ps_d[:, b, 0:390], ones_sb[:gk, :],
                                             gt[:gk, b, :],
                                             start=(gi == 0),
                                             stop=(gi == len(dgroups) - 1))
                    drecip = pat.tile([1, 2, 390], F32, tag="dr")
                    nc.vector.reciprocal_approx_fast(drecip, ps_d[:, :, 0:390])
                    drep = pat.tile([128, 2, 390], F32, tag="drep")
                    nc.gpsimd.partition_broadcast(
                        drep.rearrange("p a b -> p (a b)"),
                        drecip.rearrange("p a b -> p (a b)"))
                    at = pat.tile([128, 2, 390], BF16, tag=f"at{h}")
                    nc.vector.tensor_mul(at, ps_o[:, :, 0:390], drep)
                    attn.append(at)
                for oc in range(KC):
                    ps_oc = ppm.tile([128, 2, 512], F32, tag="pmisc")
                    for h in range(3):
                        for b in range(2):
                            nc.tensor.matmul(ps_oc[:, b, 0:390], wo_sb[:, h, oc, :],
                                             attn[h][:, b, :],
                                             start=(h == 0), stop=(h == 2))
                    o_sb = pat.tile([128, 2, 390], F16, tag="osb")
                    nc.vector.tensor_copy(o_sb, ps_oc[:, :, 0:390])
                    nc.sync.dma_start(
                        out=T["outOT"].ap()[oc * 128:(oc + 1) * 128,
                                            qf * QH:(qf + 1) * QH],
                        in_=o_sb.rearrange("p a b -> p (a b)"))


def _prep(inputs):
    g = {k: np.asarray(v) for k, v in inputs.items()}
    x = g["x"].astype(np.float32)
    assert x.shape == (1, L, D), x.shape
    has_bias = any(np.any(g[k].astype(np.float64)) for k in ("bq", "bk", "bv"))
    has_gk = not np.all(g["gk"] == 1.0)

    perm = np.empty(D, np.int64)
    for h in range(NH):
        base = h * HD
        perm[base:base + 64] = base + 2 * np.arange(64)
        perm[base + 64:base + 128] = base + 2 * np.arange(64) + 1

    cos_t, sin_t = _rope_tables(g["freqs"].astype(np.float32))  # [64, L]

    xT = np.ascontiguousarray(x[0].T).astype(BF)                # [D, L]
    xT_dev = np.ascontiguousarray(xT.reshape(KC, 128, L))

    wq_p = g["wq"].astype(np.float32)[perm]   # q feature j = row j of wq
    wk_p = g["wk"].astype(np.float32)[perm]
    gq_p = g["gq"].astype(np.float32)[perm]
    gk_p = g["gk"].astype(np.float32)[perm]

    cos_bf = cos_t.astype(BF)
    sin_bf = sin_t.astype(BF)

    in_maps = []
    qcols_all = []
    for c in range(8):
        gidx, s = c % 4, c // 4
        heads = [3 * gidx, 3 * gidx + 1, 3 * gidx + 2]
        cols = np.concatenate([np.arange(h * HD, (h + 1) * HD) for h in heads])
        qcols = np.concatenate(
            [np.arange(f * FRT + s * QH, f * FRT + (s + 1) * QH)
             for f in range(NFR)])
        qcols_all.append(qcols)

        woT = np.empty((3, KC, 128, 128), np.float32)
        for hl, h in enumerate(heads):
            for oc in range(KC):
                woT[hl, oc] = g["wo"].astype(np.float32)[
                    oc * 128:(oc + 1) * 128, h * HD:(h + 1) * HD].T
        m = dict(
            xT=xT_dev,
            xTq=np.ascontiguousarray(xT_dev[:, :, qcols]),
            wqT=np.ascontiguousarray(wq_p[cols].T).astype(BF).reshape(KC, 128, 384),
            wkT=np.ascontiguousarray(wk_p[cols].T).astype(BF).reshape(KC, 128, 384),
            wvT=np.ascontiguousarray(
                g["wv"].astype(np.float32)[cols].T).astype(BF).reshape(KC, 128, 384),
            woT=woT.astype(BF),
            cosk=cos_bf, sink=sin_bf,
            cosq=np.ascontiguousarray(cos_bf[:, qcols]),
            sinq=np.ascontiguousarray(sin_bf[:, qcols]),
            gq_d=np.ascontiguousarray(gq_p[cols].reshape(3, 128).T),
        )
        if has_gk:
            m["gk_d"] = np.ascontiguousarray(gk_p[cols].reshape(3, 128).T)
        if has_bias:
            bq_p = g["bq"].astype(np.float32)[perm]
            bk_p = g["bk"].astype(np.float32)[perm]
            m["bqkv"] = np.stack(
                [bq_p[cols], bk_p[cols], g["bv"].astype(np.float32)[cols]]
            ).reshape(1, 3, 384).astype(BF)
        m = {k: np.ascontiguousarray(v) for k, v in m.items()}
        in_maps.append(m)
    return in_maps, qcols_all, (has_bias, has_gk), g


def kernel(**inputs):
    in_maps, qcols_all, flags, g = _prep(inputs)
    if flags not in _nc_cache:
        _nc_cache[flags] = _build(*flags)
    nc = _nc_cache[flags]
    res = bass_utils.run_bass_kernel_spmd(nc, in_maps, core_ids=list(range(8)))
    out = np.zeros((D, L), np.float32)
    for c in range(8):
        out[:, qcols_all[c]] += res.results[c]["outOT"].astype(np.float32)
    out = out.T[None, :, :] + g["bo"].astype(np.float32)
    return out.astype(np.float32)
